# revision 15
# baseline (speedup 1.0000x reference)
"""Trainium2 Bass kernel for nn_DenseFlashAttention (GNN message passing).

Design: receivers are degree-sorted and packed 128 per tile-slot with their
edges along the FREE dim (slot depth D = max degree in the slot, ~1% pad).
Segment softmax/sums become native free-dim DVE reduces -- no 0/1 S-matrix
matmuls, no receiver->edge expansion matmuls.  A per-node fp16 table
[rp|tp|u_hi,v_hi|u_lo,v_lo] is built node-range-sharded in Phase A and
AllGathered; Phase B gathers one 1152B row per edge.  Receiver-side values
(u_r, decay/temp offsets, rp_r|tp_r) are computed locally from x_perm in
Phase A2 (overlaps the collective).  The weighted aggregation is an fp16
gating multiply (pair-duplicated W APs keep DVE 2x packing) + in-place
pairwise tree reduction over D.
"""
import numpy as np

import concourse.bacc as bacc
import concourse.mybir as mybir
from concourse import tile
from concourse.bass_utils import run_bass_kernel_spmd

N_NODES = 20000
N_EDGES = 200000
F = 64
H = 4
M = 32
NCORES = 8
NPC = N_NODES // NCORES           # 2500 nodes per core (range shard)
NPC_PAD = 2560                    # padded to 20 x 128
ROWW = 640                        # fp16 elems per table row (1280B, %256==0)
TROW_TOT = NPC_PAD * NCORES       # 20480
AGB = 512                         # AllGather batch rows
DUMMY_ROW_LOCAL = 2548            # shard row (core 0) holding the dummy row
DUMMY_TROW = (DUMMY_ROW_LOCAL // AGB) * (AGB * NCORES) + (DUMMY_ROW_LOCAL % AGB)

f32 = mybir.dt.float32
f32r = mybir.dt.float32r
f16 = mybir.dt.float16
i16 = mybir.dt.int16
AF = mybir.ActivationFunctionType
ALU = mybir.AluOpType
AX = mybir.AxisListType

TRACE = False          # set by test.py for NTFF profiling
TRACE_KW = {}
LAST_RESULT = {}       # exec_time_ns etc. stashed here when TRACE

_CACHE = {}


def _np_softplus(v):
    v = np.asarray(v, np.float64)
    return np.log1p(np.exp(-np.abs(v))) + np.maximum(v, 0)


def _trow(n):
    """global node id -> table row index (batched-AllGather layout)."""
    c, r = n // NPC, n % NPC
    return (r // AGB) * (AGB * NCORES) + c * AGB + (r % AGB)


def _wrap16(idx, reps=8):
    """idx [n] int -> [16, n/16] wrapped, replicated to [16*reps, n/16]."""
    n = idx.shape[0]
    assert n % 16 == 0
    w = np.ascontiguousarray(idx.reshape(n // 16, 16).T).astype(np.int16)
    return np.tile(w, (reps, 1))


def _preprocess(edge_index, edge_len):
    sender = np.asarray(edge_index[0])
    receiver = np.asarray(edge_index[1])
    el = np.asarray(edge_len, np.float32)
    deg = np.bincount(receiver, minlength=N_NODES)
    assert deg.max() <= 120, "row-major packing assumes deg <= 120"

    order = np.argsort(-deg, kind="stable")
    pos = order[deg[order] > 0]                    # positive-degree nodes
    ntg = (len(pos) + 127) // 128                  # global tiles of 128 rows
    NT = (ntg + NCORES - 1) // NCORES              # slots per core

    # edges grouped by receiver
    eorder = np.argsort(receiver, kind="stable")
    starts = np.searchsorted(receiver[eorder], np.arange(N_NODES))
    ends = np.searchsorted(receiver[eorder], np.arange(N_NODES) + 1)

    # slot depth: max degree within the slot's 8 tiles (sorted -> first tile)
    Ds = []
    for j in range(NT):
        t0 = j * NCORES
        dmax = int(deg[pos[t0 * 128]]) if t0 * 128 < len(pos) else 1
        Ds.append(max(2, dmax + (dmax & 1)))       # even, >= 2

    trow_s = np.array([_trow(int(s)) for s in sender], np.int64)

    cores = []
    for c in range(NCORES):
        idx_parts, len_parts = [], []
        node_of = np.full(NT * 128, -1, np.int64)
        for j in range(NT):
            D = Ds[j]
            g = (j * NCORES + c) * 128             # global rank offset
            rows = pos[g:g + 128]
            idx = np.full((D, 128), DUMMY_TROW, np.int64)
            lem = np.zeros((128, D), np.float32)
            for r, n in enumerate(rows):
                node_of[j * 128 + r] = n
                eids = eorder[starts[n]:ends[n]]
                d = len(eids)
                idx[:d, r] = trow_s[eids]
                lem[r, :d] = el[eids]
            idx_parts.append(idx.reshape(-1))
            len_parts.append(lem)
        flat = np.concatenate(idx_parts)
        cores.append(dict(
            g1i=_wrap16(flat),
            len_pl=np.concatenate(len_parts, axis=1),
            node_of=node_of,
        ))
    return NT, Ds, cores


def _build_program(NT, Ds, w):
    SD = int(np.sum(Ds))
    DMAX = int(max(Ds))
    ms, mb = w["mix_scale"], w["mix_bias"]
    nc = bacc.Bacc("TRN2", target_bir_lowering=False, debug=False,
                   num_devices=NCORES)

    # ---- I/O ----
    xT_d = nc.dram_tensor("xT_in", [F, NPC_PAD], f32r, kind="ExternalInput")
    xpT_d = nc.dram_tensor("xpT", [F, NT * 128], f32r, kind="ExternalInput")
    xperm_d = nc.dram_tensor("x_perm", [NT * 128, F], f32, kind="ExternalInput")
    g1i_d = nc.dram_tensor("g1i", [128, SD * 8], i16, kind="ExternalInput")
    len_d = nc.dram_tensor("len_pl", [128, SD], f32, kind="ExternalInput")
    wrt_d = nc.dram_tensor("wrt", [F, 512], f32r, kind="ExternalInput")
    wuv_d = nc.dram_tensor("wuv2", [F, 8], f32r, kind="ExternalInput")
    dw1_d = nc.dram_tensor("dw1t", [F, H * M], f32r, kind="ExternalInput")
    tw1_d = nc.dram_tensor("tw1t", [F, H * M], f32r, kind="ExternalInput")
    db1_d = nc.dram_tensor("db1c", [H * M, 1], f32, kind="ExternalInput")
    tb1_d = nc.dram_tensor("tb1c", [H * M, 1], f32, kind="ExternalInput")
    dw2_d = nc.dram_tensor("dw2bd", [H * M, H], f32, kind="ExternalInput")
    tw2_d = nc.dram_tensor("tw2bd", [H * M, H], f32, kind="ExternalInput")
    kd_d = nc.dram_tensor("kdc", [128, H], f32, kind="ExternalInput")
    kt_d = nc.dram_tensor("ktc", [128, H], f32, kind="ExternalInput")
    rtw_d = nc.dram_tensor("rtwc", [128, H], f32, kind="ExternalInput")
    wout_d = nc.dram_tensor("wout", [F, F], f32, kind="ExternalInput")
    ident_d = nc.dram_tensor("ident", [128, 128], f32, kind="ExternalInput")
    dummy_d = nc.dram_tensor("dummy_row", [1, ROWW], f16, kind="ExternalInput")
    out_d = nc.dram_tensor("out_perm", [NT * 128, F], f32, kind="ExternalOutput")

    table_shs = [nc.dram_tensor(f"table_sh{b}", [AGB, ROWW], f16)
                 for b in range(NPC_PAD // AGB)]
    table = nc.dram_tensor("table", [TROW_TOT, ROWW], f16, addr_space="Shared")

    with tile.TileContext(nc) as tc:
        # ================= Phase A: table shard =================
        with (
            tc.tile_pool(name="pa_const", bufs=1) as pac,
            tc.tile_pool(name="pa", bufs=3) as pa,
            tc.tile_pool(name="pa_ps", bufs=2, space="PSUM") as pap,
        ):
            wrt = pac.tile([F, 512], f32r)
            nc.sync.dma_start(wrt[:], wrt_d[:])
            wuv = pac.tile([F, 8], f32r)
            nc.sync.dma_start(wuv[:], wuv_d[:])
            xts = pac.tile([F, NPC_PAD], f32r)
            nc.sync.dma_start(xts[:], xT_d[:])
            drow = pac.tile([1, ROWW], f16)
            nc.sync.dma_start(drow[:], dummy_d[:])

            for b in range(NPC_PAD // 128):
                sl = slice(b * 128, (b + 1) * 128)
                ps5 = pap.tile([128, 512], f32, tag="ps5")
                nc.tensor.matmul(ps5[:], xts[:, sl], wrt[:],
                                 start=True, stop=True)
                ps8 = pap.tile([128, 8], f32, tag="ps8")
                nc.tensor.matmul(ps8[:], xts[:, sl].bitcast(f32),
                                 wuv[:].bitcast(f32), start=True, stop=True)
                rt = pa.tile([128, ROWW], f16, tag="rt")
                nc.scalar.copy(rt[:, 0:512], ps5[:])
                hi16 = pa.tile([128, 8], f16, tag="hi16")
                nc.vector.tensor_copy(hi16[:], ps8[:])
                hi32 = pa.tile([128, 8], f32, tag="hi32")
                nc.vector.tensor_copy(hi32[:], hi16[:])
                lo16 = pa.tile([128, 8], f16, tag="lo16")
                nc.vector.tensor_tensor(lo16[:], ps8[:], hi32[:],
                                        op=ALU.subtract)
                nc.vector.tensor_copy(rt[:, 512:520], hi16[:])
                nc.vector.tensor_copy(rt[:, 520:528], lo16[:])
                r0 = b * 128
                bat, boff = r0 // AGB, r0 % AGB
                nrow = 128
                if r0 <= DUMMY_ROW_LOCAL < r0 + 128:
                    nrow = DUMMY_ROW_LOCAL - r0   # leave room for dummy row
                nc.sync.dma_start(table_shs[bat][boff:boff + nrow, :],
                                  rt[0:nrow, :])
            nc.sync.dma_start(
                table_shs[DUMMY_ROW_LOCAL // AGB]
                [DUMMY_ROW_LOCAL % AGB:DUMMY_ROW_LOCAL % AGB + 1, :],
                drow[:])

        # ========= AllGather (overlaps Phase A tail + Phase A2) =========
        for b in range(NPC_PAD // AGB):
            nc.gpsimd.collective_compute(
                "AllGather", ALU.bypass,
                ins=[table_shs[b][:]],
                outs=[table[b * AGB * NCORES:(b + 1) * AGB * NCORES, :]],
                replica_groups=[list(range(NCORES))],
            )

        # ====== Phase A2: receiver-side values from x_perm (local) ======
        with tc.tile_pool(name="pc2", bufs=1) as pc2:
            xpts = pc2.tile([F, NT * 128], f32r)
            nc.sync.dma_start(xpts[:], xpT_d[:])
            wrt2 = pc2.tile([F, 512], f32r)
            nc.sync.dma_start(wrt2[:], wrt_d[:])
            wuv2 = pc2.tile([F, 8], f32r)
            nc.sync.dma_start(wuv2[:], wuv_d[:])
            dw1 = pc2.tile([F, H * M], f32r)
            nc.sync.dma_start(dw1[:], dw1_d[:])
            tw1 = pc2.tile([F, H * M], f32r)
            nc.sync.dma_start(tw1[:], tw1_d[:])
            db1 = pc2.tile([H * M, 1], f32)
            nc.sync.dma_start(db1[:], db1_d[:])
            tb1 = pc2.tile([H * M, 1], f32)
            nc.sync.dma_start(tb1[:], tb1_d[:])
            dw2 = pc2.tile([H * M, H], f32)
            nc.sync.dma_start(dw2[:], dw2_d[:])
            tw2 = pc2.tile([H * M, H], f32)
            nc.sync.dma_start(tw2[:], tw2_d[:])
            kdc = pc2.tile([128, H], f32)
            nc.sync.dma_start(kdc[:], kd_d[:])
            ktc = pc2.tile([128, H], f32)
            nc.sync.dma_start(ktc[:], kt_d[:])
            # persistent per-slot products
            rpo = [pc2.tile([128, 512], f16, tag=f"rpo{s}", name=f"rpo{s}")
                   for s in range(NT)]
            scal = [pc2.tile([128, 12], f32, tag=f"scal{s}", name=f"scal{s}")
                    for s in range(NT)]

            ngrp = (NT * 128 + 511) // 512
            with (
                tc.tile_pool(name="pa2", bufs=2) as pa2,
                tc.tile_pool(name="pa2_ps", bufs=2, space="PSUM") as p2p,
            ):
              for g in range(ngrp):
                gsl = slice(g * 512, min((g + 1) * 512, NT * 128))
                gw = gsl.stop - gsl.start
                # silu via exp + recip (keeps the whole kernel on the
                # natural_log_exp activation table -- no table swaps)
                psh1 = p2p.tile([128, 512], f32, tag="psh1")
                nc.tensor.matmul(psh1[:, 0:gw], dw1[:], xpts[:, gsl],
                                 start=True, stop=True)
                zb1 = pa2.tile([128, 512], f32, tag="zb1")
                nc.vector.tensor_tensor(
                    zb1[:, 0:gw], psh1[:, 0:gw],
                    db1[:].broadcast_to([128, gw]), op=ALU.add)
                h1 = pa2.tile([128, 512], f32, tag="h1")
                nc.scalar.activation(h1[:, 0:gw], zb1[:, 0:gw], AF.Exp,
                                     scale=-1.0)
                nc.scalar.add(h1[:, 0:gw], h1[:, 0:gw], 1.0)
                nc.vector.reciprocal(h1[:, 0:gw], h1[:, 0:gw])
                nc.vector.tensor_tensor(h1[:, 0:gw], h1[:, 0:gw],
                                        zb1[:, 0:gw], op=ALU.mult)
                psh2 = p2p.tile([128, 512], f32, tag="psh1")
                nc.tensor.matmul(psh2[:, 0:gw], tw1[:], xpts[:, gsl],
                                 start=True, stop=True)
                zb2 = pa2.tile([128, 512], f32, tag="zb2")
                nc.vector.tensor_tensor(
                    zb2[:, 0:gw], psh2[:, 0:gw],
                    tb1[:].broadcast_to([128, gw]), op=ALU.add)
                h2 = pa2.tile([128, 512], f32, tag="h2")
                nc.scalar.activation(h2[:, 0:gw], zb2[:, 0:gw], AF.Exp,
                                     scale=-1.0)
                nc.scalar.add(h2[:, 0:gw], h2[:, 0:gw], 1.0)
                nc.vector.reciprocal(h2[:, 0:gw], h2[:, 0:gw])
                nc.vector.tensor_tensor(h2[:, 0:gw], h2[:, 0:gw],
                                        zb2[:, 0:gw], op=ALU.mult)
                for k in range(gw // 128):
                    s = g * 4 + k
                    sl = slice(s * 128, (s + 1) * 128)
                    ks = slice(k * 128, (k + 1) * 128)
                    ps5b = p2p.tile([128, 512], f32, tag="ps5b")
                    nc.tensor.matmul(ps5b[:], xpts[:, sl], wrt2[:],
                                     start=True, stop=True)
                    nc.scalar.copy(rpo[s][:], ps5b[:])
                    psu = p2p.tile([128, 4], f32, tag="psu")
                    nc.tensor.matmul(psu[:], xpts[:, sl].bitcast(f32),
                                     wuv2[:, 0:4].bitcast(f32),
                                     start=True, stop=True)
                    psd = p2p.tile([128, 8], f32, tag="psd")
                    nc.tensor.matmul(psd[:, 0:4], h1[:, ks], dw2[:],
                                     start=True, stop=True)
                    nc.tensor.matmul(psd[:, 4:8], h2[:, ks], tw2[:],
                                     start=True, stop=True)
                    nc.vector.tensor_copy(scal[s][:, 0:4], psu[:])
                    nc.vector.tensor_tensor(scal[s][:, 4:8], psd[:, 0:4],
                                            kdc[:], op=ALU.add)
                    nc.vector.tensor_tensor(scal[s][:, 8:12], psd[:, 4:8],
                                            ktc[:], op=ALU.add)

            # ---- len-only planes, precomputed during the AllGather ----
            g1i = pc2.tile([128, SD * 8], i16)
            nc.sync.dma_start(g1i[:], g1i_d[:])
            lent = pc2.tile([128, SD], f32)
            nc.sync.dma_start(lent[:], len_d[:])
            rtwc = pc2.tile([128, H], f32)
            nc.sync.dma_start(rtwc[:], rtw_d[:])
            GGall = pc2.tile([128, H, SD], f32)
            TMall = pc2.tile([128, H, SD], f32)
            LBall = pc2.tile([128, H, SD], f32)
            for h in range(H):
                nc.scalar.activation(GGall[:, h, :], lent[:], AF.Exp,
                                     scale=float(-ms[h]), bias=float(-mb[h]))
            nc.vector.tensor_scalar_add(GGall[:], GGall[:], 1.0)
            nc.vector.reciprocal(GGall[:], GGall[:])
            lenb_all = lent[:].unsqueeze(1).broadcast_to([128, H, SD])
            nc.vector.tensor_tensor(
                TMall[:], lenb_all,
                rtwc[:].unsqueeze(2).broadcast_to([128, H, SD]), op=ALU.mult)
            off = 0
            for j in range(NT):
                D = Ds[j]
                lsl = slice(off, off + D)
                nc.vector.tensor_tensor(
                    TMall[:, :, lsl], TMall[:, :, lsl],
                    scal[j][:, 8:12].unsqueeze(2).broadcast_to([128, H, D]),
                    op=ALU.add)
                # LB = u_r + (rdscale+doff)*len  (subtracted from u_s later)
                nc.vector.tensor_tensor(
                    LBall[:, :, lsl],
                    lent[:, lsl].unsqueeze(1).broadcast_to([128, H, D]),
                    scal[j][:, 4:8].unsqueeze(2).broadcast_to([128, H, D]),
                    op=ALU.mult)
                nc.vector.tensor_tensor(
                    LBall[:, :, lsl], LBall[:, :, lsl],
                    scal[j][:, 0:4].unsqueeze(2).broadcast_to([128, H, D]),
                    op=ALU.add)
                off += D
            nc.scalar.activation(TMall[:], TMall[:], AF.Exp)
            nc.scalar.activation(TMall[:], TMall[:], AF.Ln, bias=1.0)
            nc.vector.tensor_scalar_add(TMall[:], TMall[:], 1e-4)
            nc.vector.reciprocal(TMall[:], TMall[:])

            # ================= Phase B =================
            with (
                tc.tile_pool(name="pbc", bufs=1) as pbc,
                tc.tile_pool(name="pbg", bufs=2) as pbg,
                tc.tile_pool(name="pf", bufs=2) as pf,
                tc.tile_pool(name="pb_ps", bufs=2, space="PSUM") as pbp,
            ):
                wout = pbc.tile([F, F], f32)
                nc.sync.dma_start(wout[:], wout_d[:])
                ident = pbc.tile([128, 128], f32)
                nc.sync.dma_start(ident[:], ident_d[:])

                off = 0
                for j in range(NT):
                    D = Ds[j]
                    lsl = slice(off, off + D)
                    G = pbg.tile([128, DMAX, ROWW], f16, tag="G")
                    nc.gpsimd.dma_gather(
                        G[:, 0:D, :], table[:, 0:ROWW],
                        g1i[:, off * 8:(off + D) * 8],
                        D * 128, D * 128, elem_size=ROWW, elem_step=ROWW,
                        single_packet=False)

                    # ---- per-edge scalars, [128, H, D] f32 planes ----
                    US = pf.tile([128, H, DMAX], f32, tag="US")
                    nc.vector.tensor_tensor(
                        US[:, :, 0:D],
                        G[:, 0:D, 512:516].rearrange("p d h -> p h d"),
                        G[:, 0:D, 520:524].rearrange("p d h -> p h d"),
                        op=ALU.add)
                    VS = pf.tile([128, H, DMAX], f32, tag="VS")
                    nc.vector.tensor_tensor(
                        VS[:, :, 0:D],
                        G[:, 0:D, 516:520].rearrange("p d h -> p h d"),
                        G[:, 0:D, 524:528].rearrange("p d h -> p h d"),
                        op=ALU.add)
                    # tangential: exp(v_s - rowmax), sum
                    MX = pf.tile([128, H], f32, tag="MX")
                    nc.vector.tensor_reduce(MX[:], VS[:, :, 0:D], AX.X,
                                            op=ALU.max)
                    nc.vector.tensor_tensor(
                        VS[:, :, 0:D], VS[:, :, 0:D],
                        MX[:].unsqueeze(2).broadcast_to([128, H, D]),
                        op=ALU.subtract)
                    ET = pf.tile([128, H, DMAX], f32, tag="ET")
                    nc.scalar.activation(ET[:, :, 0:D], VS[:, :, 0:D], AF.Exp)
                    DRT = pf.tile([128, 2 * H], f32, tag="DRT")
                    nc.vector.tensor_reduce(DRT[:, 4:8], ET[:, :, 0:D], AX.X,
                                            op=ALU.add)
                    # radial logits: (u_s - LB) / temp
                    LR = pf.tile([128, H, DMAX], f32, tag="LR")
                    nc.vector.tensor_tensor(LR[:, :, 0:D], US[:, :, 0:D],
                                            LBall[:, :, lsl], op=ALU.subtract)
                    nc.vector.tensor_tensor(LR[:, :, 0:D], LR[:, :, 0:D],
                                            TMall[:, :, lsl], op=ALU.mult)
                    nc.vector.tensor_reduce(MX[:], LR[:, :, 0:D], AX.X,
                                            op=ALU.max)
                    nc.vector.tensor_tensor(
                        LR[:, :, 0:D], LR[:, :, 0:D],
                        MX[:].unsqueeze(2).broadcast_to([128, H, D]),
                        op=ALU.subtract)
                    ER = pf.tile([128, H, DMAX], f32, tag="ER")
                    nc.scalar.activation(ER[:, :, 0:D], LR[:, :, 0:D], AF.Exp)
                    nc.vector.tensor_reduce(DRT[:, 0:4], ER[:, :, 0:D], AX.X,
                                            op=ALU.add)
                    nc.vector.reciprocal(DRT[:], DRT[:])
                    # alphas and blend weights
                    nc.vector.tensor_tensor(
                        ER[:, :, 0:D], ER[:, :, 0:D],
                        DRT[:, 0:4].unsqueeze(2).broadcast_to([128, H, D]),
                        op=ALU.mult)
                    nc.vector.tensor_tensor(
                        ET[:, :, 0:D], ET[:, :, 0:D],
                        DRT[:, 4:8].unsqueeze(2).broadcast_to([128, H, D]),
                        op=ALU.mult)
                    # BA = g*ra + (1-g)*ta = ta + g*(ra - ta)
                    GGs = GGall[:, :, lsl]
                    W1 = pf.tile([128, H, DMAX], f32, tag="W1")
                    nc.vector.tensor_tensor(W1[:, :, 0:D], ER[:, :, 0:D],
                                            ET[:, :, 0:D], op=ALU.subtract)
                    nc.vector.tensor_tensor(W1[:, :, 0:D], W1[:, :, 0:D],
                                            GGs, op=ALU.mult)
                    nc.vector.tensor_tensor(W1[:, :, 0:D], W1[:, :, 0:D],
                                            ET[:, :, 0:D], op=ALU.add)
                    # W1 <- BA*g ; W2 <- BA - BA*g
                    W2 = pf.tile([128, H, DMAX], f32, tag="W2")
                    nc.vector.tensor_tensor(W2[:, :, 0:D], W1[:, :, 0:D],
                                            GGs, op=ALU.mult)
                    nc.vector.tensor_tensor(W1[:, :, 0:D], W1[:, :, 0:D],
                                            W2[:, :, 0:D], op=ALU.subtract)
                    # now W2 holds BA*g (radial ch), W1 holds BA*(1-g) (tang)
                    C12 = pf.tile([128, 2 * H], f32, tag="C12")
                    nc.vector.tensor_reduce(C12[:, 0:4], W2[:, :, 0:D], AX.X,
                                            op=ALU.add)
                    nc.vector.tensor_reduce(C12[:, 4:8], W1[:, :, 0:D], AX.X,
                                            op=ALU.add)
                    # pack W into fp16 pair-duplicated layout [128, D, 8, 2]
                    WP = pf.tile([128, DMAX, 16], f16, tag="WP")
                    nc.vector.tensor_copy(
                        WP[:, 0:D, 0:8].rearrange("p d (c two) -> p c two d",
                                                  two=2),
                        W2[:, :, 0:D].unsqueeze(2)
                        .broadcast_to([128, H, 2, D]))
                    nc.vector.tensor_copy(
                        WP[:, 0:D, 8:16].rearrange("p d (c two) -> p c two d",
                                                   two=2),
                        W1[:, :, 0:D].unsqueeze(2)
                        .broadcast_to([128, H, 2, D]))
                    C12f = pf.tile([128, 2 * H], f16, tag="C12f")
                    nc.vector.tensor_copy(C12f[:], C12[:])

                    # ---- gating multiply into compact V + flat tree sum ----
                    V = pbg.tile([128, DMAX * 512], f16, tag="V")
                    nc.vector.tensor_tensor(
                        V[:, 0:D * 512].rearrange(
                            "p (d c g2 two) -> p d c g2 two", d=D, c=8, two=2),
                        G[:, 0:D, 0:512].rearrange(
                            "p d (c g2 two) -> p d c g2 two", c=8, two=2),
                        WP[:, 0:D, :].rearrange("p d (c two) -> p d c two",
                                                two=2)
                        .unsqueeze(3).broadcast_to([128, D, 8, 32, 2]),
                        op=ALU.mult)
                    cur = D
                    while cur > 1:
                        nxt = (cur + 1) // 2
                        cnt = cur - nxt
                        nc.vector.tensor_tensor(
                            V[:, 0:cnt * 512], V[:, 0:cnt * 512],
                            V[:, nxt * 512:cur * 512], op=ALU.add)
                        cur = nxt
                    # DIF = Vsum - [C1*rp_r | C2*tp_r]
                    DIF = pf.tile([128, 512], f16, tag="DIF")
                    nc.vector.tensor_tensor(
                        DIF[:].rearrange("p (c f) -> p c f", c=8),
                        rpo[j][:].rearrange("p (c f) -> p c f", c=8),
                        C12f[:].unsqueeze(2).broadcast_to([128, 8, F]),
                        op=ALU.mult)
                    nc.vector.tensor_tensor(DIF[:], V[:, 0:512], DIF[:],
                                            op=ALU.subtract)
                    MMt = pf.tile([128, F], f32, tag="MMt")
                    nc.vector.tensor_reduce(
                        MMt[:], DIF[:].rearrange("p (c f) -> p f c", c=8),
                        AX.X, op=ALU.add)
                    # out = x_perm + (0.25*MM) @ Wout   (0.25 folded in wout)
                    ps_tr = pbp.tile([F, 128], f32, tag="ps_tr")
                    nc.tensor.transpose(ps_tr[:], MMt[:], ident[:])
                    mT = pf.tile([F, 128], f32, tag="mT")
                    nc.vector.tensor_copy(mT[:], ps_tr[:])
                    ps_o = pbp.tile([128, F], f32, tag="ps_o")
                    nc.tensor.matmul(ps_o[:], mT[:], wout[:],
                                     start=True, stop=True)
                    xp = pbg.tile([128, F], f32, tag="xp")
                    nc.sync.dma_start(xp[:], xperm_d[j * 128:(j + 1) * 128, :])
                    ob = pf.tile([128, F], f32, tag="ob")
                    nc.vector.tensor_tensor(ob[:], ps_o[:], xp[:], op=ALU.add)
                    nc.sync.dma_start(out_d[j * 128:(j + 1) * 128, :], ob[:])
                    off += D

    nc.compile()
    return nc


def kernel(**inputs):
    x = np.asarray(inputs["x"], np.float32)
    edge_index = np.asarray(inputs["edge_index"])
    edge_len = np.asarray(inputs["edge_len"], np.float32)

    NT, Ds, cores = _preprocess(edge_index, edge_len)

    ms = np.asarray(inputs["mix_scale"], np.float32)
    mb = np.asarray(inputs["mix_bias"], np.float32)
    w = dict(mix_scale=ms, mix_bias=mb)
    key = (NT, tuple(Ds), ms.tobytes(), mb.tobytes())
    if key not in _CACHE:
        _CACHE[key] = _build_program(NT, Ds, w)
    nc = _CACHE[key]

    Wp = np.asarray(inputs["Wp"], np.float32)
    Wr = np.asarray(inputs["Wr"], np.float32)
    Wt = np.asarray(inputs["Wt"], np.float32)
    rs = np.asarray(inputs["radial_score"], np.float32)
    ts_ = np.asarray(inputs["tangential_score"], np.float32)
    wu = np.einsum("hfg,hg->fh", Wp, rs)                  # [F, H]
    wv = np.einsum("hfg,hg->fh", Wp, ts_)
    wd1f = np.einsum("hfg,hgm->fhm", Wp,
                     np.asarray(inputs["decay_W1"], np.float32))
    wt1f = np.einsum("hfg,hgm->fhm", Wp,
                     np.asarray(inputs["temp_W1"], np.float32))
    dw2bd = np.zeros((H * M, H), np.float32)
    tw2bd = np.zeros((H * M, H), np.float32)
    dw2 = np.asarray(inputs["decay_w2"], np.float32)
    tw2 = np.asarray(inputs["temp_w2"], np.float32)
    for h in range(H):
        dw2bd[h * M:(h + 1) * M, h] = dw2[h]
        tw2bd[h * M:(h + 1) * M, h] = tw2[h]
    kd = (np.asarray(inputs["decay_b2"], np.float64)
          + _np_softplus(inputs["rdls"])).astype(np.float32)
    kt = (np.asarray(inputs["temp_b2"], np.float64)
          + np.asarray(inputs["rtb"], np.float64)).astype(np.float32)
    dummy = np.zeros(ROWW, np.float16)
    dummy[512:520] = -60000.0     # u_hi, v_hi
    shared = {
        "wrt": np.ascontiguousarray(np.concatenate(
            [Wr.transpose(1, 0, 2).reshape(F, H * F),
             Wt.transpose(1, 0, 2).reshape(F, H * F)], axis=1)),
        "wuv2": np.ascontiguousarray(np.concatenate([wu, wv], axis=1)),
        "dw1t": np.ascontiguousarray(wd1f.reshape(F, H * M)),
        "tw1t": np.ascontiguousarray(wt1f.reshape(F, H * M)),
        "db1c": np.ascontiguousarray(
            np.asarray(inputs["decay_b1"], np.float32).reshape(H * M, 1)),
        "tb1c": np.ascontiguousarray(
            np.asarray(inputs["temp_b1"], np.float32).reshape(H * M, 1)),
        "dw2bd": dw2bd,
        "tw2bd": tw2bd,
        "kdc": np.tile(kd[None, :], (128, 1)),
        "ktc": np.tile(kt[None, :], (128, 1)),
        "rtwc": np.tile(np.asarray(inputs["rtw"], np.float32)[None, :],
                        (128, 1)),
        "wout": np.ascontiguousarray(
            0.25 * np.asarray(inputs["Wout"], np.float32)),
        "ident": np.eye(128, dtype=np.float32),
        "dummy_row": dummy[None, :],
    }

    in_maps = []
    for c in range(NCORES):
        cc = cores[c]
        xr = np.zeros((F, NPC_PAD), np.float32)
        xr[:, :NPC] = x[c * NPC:(c + 1) * NPC].T
        xp = np.zeros((NT * 128, F), np.float32)
        valid = cc["node_of"] >= 0
        xp[valid] = x[cc["node_of"][valid]]
        in_maps.append(dict(shared, xT_in=xr, x_perm=xp,
                            xpT=np.ascontiguousarray(xp.T),
                            g1i=cc["g1i"], len_pl=cc["len_pl"]))

    r = run_bass_kernel_spmd(nc, in_maps, list(range(NCORES)),
                             trace=TRACE, **TRACE_KW)
    if TRACE:
        LAST_RESULT["exec_time_ns"] = r.exec_time_ns
        LAST_RESULT["mean_exec_time_ns"] = r.mean_exec_time_ns
        LAST_RESULT["raw"] = r

    out = np.array(x, np.float32, copy=True)  # zero-degree nodes: out = x
    for c in range(NCORES):
        cc = cores[c]
        rows = r.results[c]["out_perm"]
        valid = cc["node_of"] >= 0
        out[cc["node_of"][valid]] = rows[valid]
    return out


# revision 23
# speedup vs baseline: 1.2140x; 1.2140x over previous
"""Trainium2 Bass kernel for nn_DenseFlashAttention (GNN message passing).

Design: receivers are degree-sorted and packed 128 per tile-slot with their
edges along the FREE dim (slot depth D = max degree in the slot, ~1% pad).
Segment softmax/sums become native free-dim DVE reduces -- no 0/1 S-matrix
matmuls, no receiver->edge expansion matmuls.  A per-node fp16 table
[rp|tp|u_hi,v_hi|u_lo,v_lo] is built node-range-sharded in Phase A and
AllGathered; Phase B gathers one 1152B row per edge.  Receiver-side values
(u_r, decay/temp offsets, rp_r|tp_r) are computed locally from x_perm in
Phase A2 (overlaps the collective).  The weighted aggregation is an fp16
gating multiply (pair-duplicated W APs keep DVE 2x packing) + in-place
pairwise tree reduction over D.
"""
import numpy as np

import concourse.bacc as bacc
import concourse.mybir as mybir
from concourse import tile
from concourse.bass_utils import run_bass_kernel_spmd

N_NODES = 20000
N_EDGES = 200000
F = 64
H = 4
M = 32
NCORES = 8
NPC = N_NODES // NCORES           # 2500 nodes per core (range shard)
NPC_PAD = 2560                    # padded to 20 x 128
ROWW = 640                        # fp16 elems per table row (1280B, %256==0)
TROW_TOT = NPC_PAD * NCORES       # 20480
AGB = 512                         # AllGather batch rows
DUMMY_ROW_LOCAL = 2548            # shard row (core 0) holding the dummy row
DUMMY_TROW = (DUMMY_ROW_LOCAL // AGB) * (AGB * NCORES) + (DUMMY_ROW_LOCAL % AGB)

f32 = mybir.dt.float32
f32r = mybir.dt.float32r
f16 = mybir.dt.float16
i16 = mybir.dt.int16
AF = mybir.ActivationFunctionType
ALU = mybir.AluOpType
AX = mybir.AxisListType

TRACE = False          # set by test.py for NTFF profiling
TRACE_KW = {}
LAST_RESULT = {}       # exec_time_ns etc. stashed here when TRACE

_CACHE = {}


def _np_softplus(v):
    v = np.asarray(v, np.float64)
    return np.log1p(np.exp(-np.abs(v))) + np.maximum(v, 0)


def _trow(n):
    """global node id -> table row index (batched-AllGather layout)."""
    c, r = n // NPC, n % NPC
    return (r // AGB) * (AGB * NCORES) + c * AGB + (r % AGB)


def _wrap16(idx, reps=8):
    """idx [n] int -> [16, n/16] wrapped, replicated to [16*reps, n/16]."""
    n = idx.shape[0]
    assert n % 16 == 0
    w = np.ascontiguousarray(idx.reshape(n // 16, 16).T).astype(np.int16)
    return np.tile(w, (reps, 1))


def _preprocess(edge_index, edge_len):
    sender = np.asarray(edge_index[0])
    receiver = np.asarray(edge_index[1])
    el = np.asarray(edge_len, np.float32)
    deg = np.bincount(receiver, minlength=N_NODES)
    assert deg.max() <= 120, "row-major packing assumes deg <= 120"

    order = np.argsort(-deg, kind="stable")
    pos = order[deg[order] > 0]                    # positive-degree nodes
    ntg = (len(pos) + 127) // 128                  # global tiles of 128 rows
    NT = (ntg + NCORES - 1) // NCORES              # slots per core

    # edges grouped by receiver
    eorder = np.argsort(receiver, kind="stable")
    starts = np.searchsorted(receiver[eorder], np.arange(N_NODES))
    ends = np.searchsorted(receiver[eorder], np.arange(N_NODES) + 1)

    # slot depth: max degree within the slot's 8 tiles (sorted -> first tile)
    Ds = []
    for j in range(NT):
        t0 = j * NCORES
        dmax = int(deg[pos[t0 * 128]]) if t0 * 128 < len(pos) else 1
        Ds.append(max(2, dmax + (dmax & 1)))       # even, >= 2

    trow_s = np.array([_trow(int(s)) for s in sender], np.int64)

    cores = []
    for c in range(NCORES):
        idx_parts, len_parts = [], []
        node_of = np.full(NT * 128, -1, np.int64)
        for j in range(NT):
            D = Ds[j]
            g = (j * NCORES + c) * 128             # global rank offset
            rows = pos[g:g + 128]
            idx = np.full((D, 128), DUMMY_TROW, np.int64)
            lem = np.zeros((128, D), np.float32)
            for r, n in enumerate(rows):
                node_of[j * 128 + r] = n
                eids = eorder[starts[n]:ends[n]]
                d = len(eids)
                idx[:d, r] = trow_s[eids]
                lem[r, :d] = el[eids]
            idx_parts.append(idx.reshape(-1))
            len_parts.append(lem)
        flat = np.concatenate(idx_parts)
        cores.append(dict(
            g1i=_wrap16(flat),
            len_pl=np.concatenate(len_parts, axis=1),
            node_of=node_of,
        ))
    return NT, Ds, cores


def _build_program(NT, Ds, w):
    SD = int(np.sum(Ds))
    DMAX = int(max(Ds))
    ms, mb = w["mix_scale"], w["mix_bias"]
    nc = bacc.Bacc("TRN2", target_bir_lowering=False, debug=False,
                   num_devices=NCORES)

    # ---- I/O ----
    xT_d = nc.dram_tensor("xT_in", [F, NPC_PAD], f32r, kind="ExternalInput")
    xpT_d = nc.dram_tensor("xpT", [F, NT * 128], f32r, kind="ExternalInput")
    xperm_d = nc.dram_tensor("x_perm", [NT * 128, F], f32, kind="ExternalInput")
    g1i_d = nc.dram_tensor("g1i", [128, SD * 8], i16, kind="ExternalInput")
    len_d = nc.dram_tensor("len_pl", [128, SD], f32, kind="ExternalInput")
    wrt_d = nc.dram_tensor("wrt", [F, 512], f32r, kind="ExternalInput")
    wuv_d = nc.dram_tensor("wuv2", [F, 8], f32r, kind="ExternalInput")
    dw1_d = nc.dram_tensor("dw1t", [F, H * M], f32r, kind="ExternalInput")
    tw1_d = nc.dram_tensor("tw1t", [F, H * M], f32r, kind="ExternalInput")
    db1_d = nc.dram_tensor("db1c", [H * M, 1], f32, kind="ExternalInput")
    tb1_d = nc.dram_tensor("tb1c", [H * M, 1], f32, kind="ExternalInput")
    dw2_d = nc.dram_tensor("dw2bd", [H * M, H], f32, kind="ExternalInput")
    tw2_d = nc.dram_tensor("tw2bd", [H * M, H], f32, kind="ExternalInput")
    kd_d = nc.dram_tensor("kdc", [128, H], f32, kind="ExternalInput")
    kt_d = nc.dram_tensor("ktc", [128, H], f32, kind="ExternalInput")
    rtw_d = nc.dram_tensor("rtwc", [128, H], f32, kind="ExternalInput")
    wout_d = nc.dram_tensor("wout", [F, F], f32, kind="ExternalInput")
    ident_d = nc.dram_tensor("ident", [128, 128], f32, kind="ExternalInput")
    dummy_d = nc.dram_tensor("dummy_row", [1, ROWW], f16, kind="ExternalInput")
    out_d = nc.dram_tensor("out_perm", [NT * 128, F], f32, kind="ExternalOutput")

    table_shs = [nc.dram_tensor(f"table_sh{b}", [AGB, ROWW], f16)
                 for b in range(NPC_PAD // AGB)]
    table = nc.dram_tensor("table", [TROW_TOT, ROWW], f16, addr_space="Shared")

    with tile.TileContext(nc) as tc:
        # ================= Phase A: table shard =================
        with (
            tc.tile_pool(name="pa_const", bufs=1) as pac,
            tc.tile_pool(name="pa", bufs=3) as pa,
            tc.tile_pool(name="pa_ps", bufs=2, space="PSUM") as pap,
        ):
            wrt = pac.tile([F, 512], f32r)
            nc.sync.dma_start(wrt[:], wrt_d[:])
            wuv = pac.tile([F, 8], f32r)
            nc.sync.dma_start(wuv[:], wuv_d[:])
            xts = pac.tile([F, NPC_PAD], f32r)
            nc.sync.dma_start(xts[:], xT_d[:])
            drow = pac.tile([1, ROWW], f16)
            nc.sync.dma_start(drow[:], dummy_d[:])

            for b in range(NPC_PAD // 128):
                sl = slice(b * 128, (b + 1) * 128)
                ps5 = pap.tile([128, 512], f32, tag="ps5")
                nc.tensor.matmul(ps5[:], xts[:, sl], wrt[:],
                                 start=True, stop=True)
                ps8 = pap.tile([128, 8], f32, tag="ps8")
                nc.tensor.matmul(ps8[:], xts[:, sl].bitcast(f32),
                                 wuv[:].bitcast(f32), start=True, stop=True)
                rt = pa.tile([128, ROWW], f16, tag="rt")
                nc.scalar.copy(rt[:, 0:512], ps5[:])
                # u,v stored as raw f32 bitcast into fp16 slots 512:528
                nc.scalar.copy(rt[:, 512:528].bitcast(f32), ps8[:])
                r0 = b * 128
                bat, boff = r0 // AGB, r0 % AGB
                nrow = 128
                if r0 <= DUMMY_ROW_LOCAL < r0 + 128:
                    nrow = DUMMY_ROW_LOCAL - r0   # leave room for dummy row
                nc.sync.dma_start(table_shs[bat][boff:boff + nrow, :],
                                  rt[0:nrow, :])
            nc.sync.dma_start(
                table_shs[DUMMY_ROW_LOCAL // AGB]
                [DUMMY_ROW_LOCAL % AGB:DUMMY_ROW_LOCAL % AGB + 1, :],
                drow[:])

        # ========= AllGather (overlaps Phase A tail + Phase A2) =========
        for b in range(NPC_PAD // AGB):
            nc.gpsimd.collective_compute(
                "AllGather", ALU.bypass,
                ins=[table_shs[b][:]],
                outs=[table[b * AGB * NCORES:(b + 1) * AGB * NCORES, :]],
                replica_groups=[list(range(NCORES))],
            )

        # ====== Phase A2: receiver-side values from x_perm (local) ======
        with tc.tile_pool(name="pc2", bufs=1) as pc2:
            xpts = pc2.tile([F, NT * 128], f32r)
            nc.sync.dma_start(xpts[:], xpT_d[:])
            wrt2 = pc2.tile([F, 512], f32r)
            nc.sync.dma_start(wrt2[:], wrt_d[:])
            wuv2 = pc2.tile([F, 8], f32r)
            nc.sync.dma_start(wuv2[:], wuv_d[:])
            dw1 = pc2.tile([F, H * M], f32r)
            nc.sync.dma_start(dw1[:], dw1_d[:])
            tw1 = pc2.tile([F, H * M], f32r)
            nc.sync.dma_start(tw1[:], tw1_d[:])
            db1 = pc2.tile([H * M, 1], f32)
            nc.sync.dma_start(db1[:], db1_d[:])
            tb1 = pc2.tile([H * M, 1], f32)
            nc.sync.dma_start(tb1[:], tb1_d[:])
            dw2 = pc2.tile([H * M, H], f32)
            nc.sync.dma_start(dw2[:], dw2_d[:])
            tw2 = pc2.tile([H * M, H], f32)
            nc.sync.dma_start(tw2[:], tw2_d[:])
            kdc = pc2.tile([128, H], f32)
            nc.sync.dma_start(kdc[:], kd_d[:])
            ktc = pc2.tile([128, H], f32)
            nc.sync.dma_start(ktc[:], kt_d[:])
            # persistent per-slot products
            rpo = [pc2.tile([128, 512], f16, tag=f"rpo{s}", name=f"rpo{s}")
                   for s in range(NT)]
            scal = [pc2.tile([128, 12], f32, tag=f"scal{s}", name=f"scal{s}")
                    for s in range(NT)]

            ngrp = (NT * 128 + 511) // 512
            with (
                tc.tile_pool(name="pa2", bufs=2) as pa2,
                tc.tile_pool(name="pa2_ps", bufs=2, space="PSUM") as p2p,
            ):
              for g in range(ngrp):
                gsl = slice(g * 512, min((g + 1) * 512, NT * 128))
                gw = gsl.stop - gsl.start
                # silu via exp + recip (keeps the whole kernel on the
                # natural_log_exp activation table -- no table swaps)
                psh1 = p2p.tile([128, 512], f32, tag="psh1")
                nc.tensor.matmul(psh1[:, 0:gw], dw1[:], xpts[:, gsl],
                                 start=True, stop=True)
                zb1 = pa2.tile([128, 512], f32, tag="zb1")
                nc.vector.tensor_tensor(
                    zb1[:, 0:gw], psh1[:, 0:gw],
                    db1[:].broadcast_to([128, gw]), op=ALU.add)
                h1 = pa2.tile([128, 512], f32, tag="h1")
                nc.scalar.activation(h1[:, 0:gw], zb1[:, 0:gw], AF.Exp,
                                     scale=-1.0)
                nc.scalar.add(h1[:, 0:gw], h1[:, 0:gw], 1.0)
                nc.vector.reciprocal(h1[:, 0:gw], h1[:, 0:gw])
                nc.vector.tensor_tensor(h1[:, 0:gw], h1[:, 0:gw],
                                        zb1[:, 0:gw], op=ALU.mult)
                psh2 = p2p.tile([128, 512], f32, tag="psh1")
                nc.tensor.matmul(psh2[:, 0:gw], tw1[:], xpts[:, gsl],
                                 start=True, stop=True)
                zb2 = pa2.tile([128, 512], f32, tag="zb2")
                nc.vector.tensor_tensor(
                    zb2[:, 0:gw], psh2[:, 0:gw],
                    tb1[:].broadcast_to([128, gw]), op=ALU.add)
                h2 = pa2.tile([128, 512], f32, tag="h2")
                nc.scalar.activation(h2[:, 0:gw], zb2[:, 0:gw], AF.Exp,
                                     scale=-1.0)
                nc.scalar.add(h2[:, 0:gw], h2[:, 0:gw], 1.0)
                nc.vector.reciprocal(h2[:, 0:gw], h2[:, 0:gw])
                nc.vector.tensor_tensor(h2[:, 0:gw], h2[:, 0:gw],
                                        zb2[:, 0:gw], op=ALU.mult)
                for k in range(gw // 128):
                    s = g * 4 + k
                    sl = slice(s * 128, (s + 1) * 128)
                    ks = slice(k * 128, (k + 1) * 128)
                    ps5b = p2p.tile([128, 512], f32, tag="ps5b")
                    nc.tensor.matmul(ps5b[:], xpts[:, sl], wrt2[:],
                                     start=True, stop=True)
                    nc.scalar.copy(rpo[s][:], ps5b[:])
                    psu = p2p.tile([128, 4], f32, tag="psu")
                    nc.tensor.matmul(psu[:], xpts[:, sl].bitcast(f32),
                                     wuv2[:, 0:4].bitcast(f32),
                                     start=True, stop=True)
                    psd = p2p.tile([128, 8], f32, tag="psd")
                    nc.tensor.matmul(psd[:, 0:4], h1[:, ks], dw2[:],
                                     start=True, stop=True)
                    nc.tensor.matmul(psd[:, 4:8], h2[:, ks], tw2[:],
                                     start=True, stop=True)
                    nc.vector.tensor_copy(scal[s][:, 0:4], psu[:])
                    nc.vector.tensor_tensor(scal[s][:, 4:8], psd[:, 0:4],
                                            kdc[:], op=ALU.add)
                    nc.vector.tensor_tensor(scal[s][:, 8:12], psd[:, 4:8],
                                            ktc[:], op=ALU.add)

            # ---- len-only planes, precomputed during the AllGather ----
            g1i = pc2.tile([128, SD * 8], i16)
            nc.sync.dma_start(g1i[:], g1i_d[:])
            lent = pc2.tile([128, SD], f32)
            nc.sync.dma_start(lent[:], len_d[:])
            rtwc = pc2.tile([128, H], f32)
            nc.sync.dma_start(rtwc[:], rtw_d[:])
            GGall = pc2.tile([128, H, SD], f32)
            TMall = pc2.tile([128, H, SD], f32)
            LBall = pc2.tile([128, H, SD], f32)
            for h in range(H):
                nc.scalar.activation(GGall[:, h, :], lent[:], AF.Exp,
                                     scale=float(-ms[h]), bias=float(-mb[h]))
            nc.vector.tensor_scalar_add(GGall[:], GGall[:], 1.0)
            nc.vector.reciprocal(GGall[:], GGall[:])
            lenb_all = lent[:].unsqueeze(1).broadcast_to([128, H, SD])
            nc.vector.tensor_tensor(
                TMall[:], lenb_all,
                rtwc[:].unsqueeze(2).broadcast_to([128, H, SD]), op=ALU.mult)
            off = 0
            for j in range(NT):
                D = Ds[j]
                lsl = slice(off, off + D)
                nc.vector.tensor_tensor(
                    TMall[:, :, lsl], TMall[:, :, lsl],
                    scal[j][:, 8:12].unsqueeze(2).broadcast_to([128, H, D]),
                    op=ALU.add)
                # LB = u_r + (rdscale+doff)*len  (subtracted from u_s later)
                nc.vector.tensor_tensor(
                    LBall[:, :, lsl],
                    lent[:, lsl].unsqueeze(1).broadcast_to([128, H, D]),
                    scal[j][:, 4:8].unsqueeze(2).broadcast_to([128, H, D]),
                    op=ALU.mult)
                nc.vector.tensor_tensor(
                    LBall[:, :, lsl], LBall[:, :, lsl],
                    scal[j][:, 0:4].unsqueeze(2).broadcast_to([128, H, D]),
                    op=ALU.add)
                off += D
            nc.scalar.activation(TMall[:], TMall[:], AF.Exp)
            nc.scalar.activation(TMall[:], TMall[:], AF.Ln, bias=1.0)
            nc.vector.tensor_scalar_add(TMall[:], TMall[:], 1e-4)
            nc.vector.reciprocal(TMall[:], TMall[:])

            # ================= Phase B =================
            with (
                tc.tile_pool(name="pbc", bufs=1) as pbc,
                tc.tile_pool(name="pbg", bufs=2) as pbg,
                tc.tile_pool(name="pf", bufs=2) as pf,
                tc.tile_pool(name="pb_ps", bufs=2, space="PSUM") as pbp,
            ):
                wout = pbc.tile([F, F], f32)
                nc.sync.dma_start(wout[:], wout_d[:])
                ident = pbc.tile([128, 128], f32)
                nc.sync.dma_start(ident[:], ident_d[:])

                off = 0
                for j in range(NT):
                    D = Ds[j]
                    lsl = slice(off, off + D)
                    G = pbg.tile([128, DMAX, ROWW], f16, tag="G", bufs=3)
                    nc.gpsimd.dma_gather(
                        G[:, 0:D, :], table[:, 0:ROWW],
                        g1i[:, off * 8:(off + D) * 8],
                        D * 128, D * 128, elem_size=ROWW, elem_step=ROWW,
                        single_packet=False)

                    # ---- per-edge scalars, [128, H, D] f32 planes ----
                    # u,v of senders live as bitcast f32 in row slots 512:528
                    USv = G[:, 0:D, 512:520].bitcast(f32) \
                        .rearrange("p d h -> p h d")
                    VSv = G[:, 0:D, 520:528].bitcast(f32) \
                        .rearrange("p d h -> p h d")
                    # tangential: exp(v_s - rowmax), sum
                    MX = pf.tile([128, H], f32, tag="MX")
                    nc.vector.tensor_reduce(MX[:], VSv, AX.X, op=ALU.max)
                    LT = pf.tile([128, H, DMAX], f32, tag="LT")
                    nc.vector.tensor_tensor(
                        LT[:, :, 0:D], VSv,
                        MX[:].unsqueeze(2).broadcast_to([128, H, D]),
                        op=ALU.subtract)
                    ET = pf.tile([128, H, DMAX], f32, tag="ET")
                    nc.scalar.activation(ET[:, :, 0:D], LT[:, :, 0:D], AF.Exp)
                    DRT = pf.tile([128, 2 * H], f32, tag="DRT")
                    nc.vector.tensor_reduce(DRT[:, 4:8], ET[:, :, 0:D], AX.X,
                                            op=ALU.add)
                    # radial logits: (u_s - LB) / temp
                    LR = pf.tile([128, H, DMAX], f32, tag="LR")
                    nc.vector.tensor_tensor(LR[:, :, 0:D], USv,
                                            LBall[:, :, lsl], op=ALU.subtract)
                    nc.vector.tensor_tensor(LR[:, :, 0:D], LR[:, :, 0:D],
                                            TMall[:, :, lsl], op=ALU.mult)
                    nc.vector.tensor_reduce(MX[:], LR[:, :, 0:D], AX.X,
                                            op=ALU.max)
                    nc.vector.tensor_tensor(
                        LR[:, :, 0:D], LR[:, :, 0:D],
                        MX[:].unsqueeze(2).broadcast_to([128, H, D]),
                        op=ALU.subtract)
                    ER = pf.tile([128, H, DMAX], f32, tag="ER")
                    nc.scalar.activation(ER[:, :, 0:D], LR[:, :, 0:D], AF.Exp)
                    nc.vector.tensor_reduce(DRT[:, 0:4], ER[:, :, 0:D], AX.X,
                                            op=ALU.add)
                    nc.vector.reciprocal(DRT[:], DRT[:])
                    # alphas and blend weights
                    nc.vector.tensor_tensor(
                        ER[:, :, 0:D], ER[:, :, 0:D],
                        DRT[:, 0:4].unsqueeze(2).broadcast_to([128, H, D]),
                        op=ALU.mult)
                    nc.vector.tensor_tensor(
                        ET[:, :, 0:D], ET[:, :, 0:D],
                        DRT[:, 4:8].unsqueeze(2).broadcast_to([128, H, D]),
                        op=ALU.mult)
                    # BA = g*ra + (1-g)*ta = ta + g*(ra - ta)
                    GGs = GGall[:, :, lsl]
                    W1 = pf.tile([128, H, DMAX], f32, tag="W1")
                    nc.vector.tensor_tensor(W1[:, :, 0:D], ER[:, :, 0:D],
                                            ET[:, :, 0:D], op=ALU.subtract)
                    nc.vector.tensor_tensor(W1[:, :, 0:D], W1[:, :, 0:D],
                                            GGs, op=ALU.mult)
                    nc.vector.tensor_tensor(W1[:, :, 0:D], W1[:, :, 0:D],
                                            ET[:, :, 0:D], op=ALU.add)
                    # W1 <- BA*g ; W2 <- BA - BA*g
                    W2 = pf.tile([128, H, DMAX], f32, tag="W2")
                    nc.vector.tensor_tensor(W2[:, :, 0:D], W1[:, :, 0:D],
                                            GGs, op=ALU.mult)
                    nc.vector.tensor_tensor(W1[:, :, 0:D], W1[:, :, 0:D],
                                            W2[:, :, 0:D], op=ALU.subtract)
                    # now W2 holds BA*g (radial ch), W1 holds BA*(1-g) (tang)
                    C12 = pf.tile([128, 2 * H], f32, tag="C12")
                    nc.vector.tensor_reduce(C12[:, 0:4], W2[:, :, 0:D], AX.X,
                                            op=ALU.add)
                    nc.vector.tensor_reduce(C12[:, 4:8], W1[:, :, 0:D], AX.X,
                                            op=ALU.add)
                    # pack W into fp16 pair-duplicated layout [128, D, 8, 2]
                    WP = pf.tile([128, DMAX, 16], f16, tag="WP")
                    nc.scalar.copy(
                        WP[:, 0:D, 0:8].rearrange("p d (c two) -> p c two d",
                                                  two=2),
                        W2[:, :, 0:D].unsqueeze(2)
                        .broadcast_to([128, H, 2, D]))
                    nc.scalar.copy(
                        WP[:, 0:D, 8:16].rearrange("p d (c two) -> p c two d",
                                                   two=2),
                        W1[:, :, 0:D].unsqueeze(2)
                        .broadcast_to([128, H, 2, D]))
                    C12f = pf.tile([128, 2 * H], f16, tag="C12f")
                    nc.scalar.copy(C12f[:], C12[:])

                    # ---- gating multiply into compact V + flat tree sum ----
                    V = pbg.tile([128, DMAX * 512], f16, tag="V", bufs=1)
                    nc.vector.tensor_tensor(
                        V[:, 0:D * 512].rearrange(
                            "p (d c g2 two) -> p d c g2 two", d=D, c=8, two=2),
                        G[:, 0:D, 0:512].rearrange(
                            "p d (c g2 two) -> p d c g2 two", c=8, two=2),
                        WP[:, 0:D, :].rearrange("p d (c two) -> p d c two",
                                                two=2)
                        .unsqueeze(3).broadcast_to([128, D, 8, 32, 2]),
                        op=ALU.mult)
                    cur = D
                    while cur > 1:
                        nxt = (cur + 1) // 2
                        cnt = cur - nxt
                        nc.vector.tensor_tensor(
                            V[:, 0:cnt * 512], V[:, 0:cnt * 512],
                            V[:, nxt * 512:cur * 512], op=ALU.add)
                        cur = nxt
                    # DIF = Vsum - [C1*rp_r | C2*tp_r]
                    DIF = pf.tile([128, 512], f16, tag="DIF")
                    nc.vector.tensor_tensor(
                        DIF[:].rearrange("p (c f) -> p c f", c=8),
                        rpo[j][:].rearrange("p (c f) -> p c f", c=8),
                        C12f[:].unsqueeze(2).broadcast_to([128, 8, F]),
                        op=ALU.mult)
                    nc.vector.tensor_tensor(DIF[:], V[:, 0:512], DIF[:],
                                            op=ALU.subtract)
                    MMt = pf.tile([128, F], f32, tag="MMt")
                    nc.vector.tensor_reduce(
                        MMt[:], DIF[:].rearrange("p (c f) -> p f c", c=8),
                        AX.X, op=ALU.add)
                    # out = x_perm + (0.25*MM) @ Wout   (0.25 folded in wout)
                    ps_tr = pbp.tile([F, 128], f32, tag="ps_tr")
                    nc.tensor.transpose(ps_tr[:], MMt[:], ident[:])
                    mT = pf.tile([F, 128], f32, tag="mT")
                    nc.scalar.copy(mT[:], ps_tr[:])
                    ps_o = pbp.tile([128, F], f32, tag="ps_o")
                    nc.tensor.matmul(ps_o[:], mT[:], wout[:],
                                     start=True, stop=True)
                    xp = pbg.tile([128, F], f32, tag="xp")
                    nc.sync.dma_start(xp[:], xperm_d[j * 128:(j + 1) * 128, :])
                    ob = pf.tile([128, F], f32, tag="ob")
                    nc.vector.tensor_tensor(ob[:], ps_o[:], xp[:], op=ALU.add)
                    nc.sync.dma_start(out_d[j * 128:(j + 1) * 128, :], ob[:])
                    off += D

    nc.compile()
    return nc


def kernel(**inputs):
    x = np.asarray(inputs["x"], np.float32)
    edge_index = np.asarray(inputs["edge_index"])
    edge_len = np.asarray(inputs["edge_len"], np.float32)

    NT, Ds, cores = _preprocess(edge_index, edge_len)

    ms = np.asarray(inputs["mix_scale"], np.float32)
    mb = np.asarray(inputs["mix_bias"], np.float32)
    w = dict(mix_scale=ms, mix_bias=mb)
    key = (NT, tuple(Ds), ms.tobytes(), mb.tobytes())
    if key not in _CACHE:
        _CACHE[key] = _build_program(NT, Ds, w)
    nc = _CACHE[key]

    Wp = np.asarray(inputs["Wp"], np.float32)
    Wr = np.asarray(inputs["Wr"], np.float32)
    Wt = np.asarray(inputs["Wt"], np.float32)
    rs = np.asarray(inputs["radial_score"], np.float32)
    ts_ = np.asarray(inputs["tangential_score"], np.float32)
    wu = np.einsum("hfg,hg->fh", Wp, rs)                  # [F, H]
    wv = np.einsum("hfg,hg->fh", Wp, ts_)
    wd1f = np.einsum("hfg,hgm->fhm", Wp,
                     np.asarray(inputs["decay_W1"], np.float32))
    wt1f = np.einsum("hfg,hgm->fhm", Wp,
                     np.asarray(inputs["temp_W1"], np.float32))
    dw2bd = np.zeros((H * M, H), np.float32)
    tw2bd = np.zeros((H * M, H), np.float32)
    dw2 = np.asarray(inputs["decay_w2"], np.float32)
    tw2 = np.asarray(inputs["temp_w2"], np.float32)
    for h in range(H):
        dw2bd[h * M:(h + 1) * M, h] = dw2[h]
        tw2bd[h * M:(h + 1) * M, h] = tw2[h]
    kd = (np.asarray(inputs["decay_b2"], np.float64)
          + _np_softplus(inputs["rdls"])).astype(np.float32)
    kt = (np.asarray(inputs["temp_b2"], np.float64)
          + np.asarray(inputs["rtb"], np.float64)).astype(np.float32)
    dummy = np.zeros(ROWW, np.float16)
    # u,v are stored as raw f32 bitcast into fp16 slots 512:528
    dummy[512:528].view(np.float32)[:] = -60000.0
    shared = {
        "wrt": np.ascontiguousarray(np.concatenate(
            [Wr.transpose(1, 0, 2).reshape(F, H * F),
             Wt.transpose(1, 0, 2).reshape(F, H * F)], axis=1)),
        "wuv2": np.ascontiguousarray(np.concatenate([wu, wv], axis=1)),
        "dw1t": np.ascontiguousarray(wd1f.reshape(F, H * M)),
        "tw1t": np.ascontiguousarray(wt1f.reshape(F, H * M)),
        "db1c": np.ascontiguousarray(
            np.asarray(inputs["decay_b1"], np.float32).reshape(H * M, 1)),
        "tb1c": np.ascontiguousarray(
            np.asarray(inputs["temp_b1"], np.float32).reshape(H * M, 1)),
        "dw2bd": dw2bd,
        "tw2bd": tw2bd,
        "kdc": np.tile(kd[None, :], (128, 1)),
        "ktc": np.tile(kt[None, :], (128, 1)),
        "rtwc": np.tile(np.asarray(inputs["rtw"], np.float32)[None, :],
                        (128, 1)),
        "wout": np.ascontiguousarray(
            0.25 * np.asarray(inputs["Wout"], np.float32)),
        "ident": np.eye(128, dtype=np.float32),
        "dummy_row": dummy[None, :],
    }

    in_maps = []
    for c in range(NCORES):
        cc = cores[c]
        xr = np.zeros((F, NPC_PAD), np.float32)
        xr[:, :NPC] = x[c * NPC:(c + 1) * NPC].T
        xp = np.zeros((NT * 128, F), np.float32)
        valid = cc["node_of"] >= 0
        xp[valid] = x[cc["node_of"][valid]]
        in_maps.append(dict(shared, xT_in=xr, x_perm=xp,
                            xpT=np.ascontiguousarray(xp.T),
                            g1i=cc["g1i"], len_pl=cc["len_pl"]))

    r = run_bass_kernel_spmd(nc, in_maps, list(range(NCORES)),
                             trace=TRACE, **TRACE_KW)
    if TRACE:
        LAST_RESULT["exec_time_ns"] = r.exec_time_ns
        LAST_RESULT["mean_exec_time_ns"] = r.mean_exec_time_ns
        LAST_RESULT["raw"] = r

    out = np.array(x, np.float32, copy=True)  # zero-degree nodes: out = x
    for c in range(NCORES):
        cc = cores[c]
        rows = r.results[c]["out_perm"]
        valid = cc["node_of"] >= 0
        out[cc["node_of"][valid]] = rows[valid]
    return out


# revision 24
# speedup vs baseline: 1.2557x; 1.0343x over previous
"""Trainium2 Bass kernel for nn_DenseFlashAttention (GNN message passing).

Design: receivers are degree-sorted and packed 128 per tile-slot with their
edges along the FREE dim (slot depth D = max degree in the slot, ~1% pad).
Segment softmax/sums become native free-dim DVE reduces -- no 0/1 S-matrix
matmuls, no receiver->edge expansion matmuls.  A per-node fp16 table
[rp|tp|u_hi,v_hi|u_lo,v_lo] is built node-range-sharded in Phase A and
AllGathered; Phase B gathers one 1152B row per edge.  Receiver-side values
(u_r, decay/temp offsets, rp_r|tp_r) are computed locally from x_perm in
Phase A2 (overlaps the collective).  The weighted aggregation is an fp16
gating multiply (pair-duplicated W APs keep DVE 2x packing) + in-place
pairwise tree reduction over D.
"""
import numpy as np

import concourse.bacc as bacc
import concourse.mybir as mybir
from concourse import tile
from concourse.bass_utils import run_bass_kernel_spmd

N_NODES = 20000
N_EDGES = 200000
F = 64
H = 4
M = 32
NCORES = 8
NPC = N_NODES // NCORES           # 2500 nodes per core (range shard)
NPC_PAD = 2560                    # padded to 20 x 128
ROWW = 640                        # fp16 elems per table row (1280B, %256==0)
TROW_TOT = NPC_PAD * NCORES       # 20480
AGB = 1280                        # AllGather batch rows
DUMMY_ROW_LOCAL = 2548            # shard row (core 0) holding the dummy row
DUMMY_TROW = (DUMMY_ROW_LOCAL // AGB) * (AGB * NCORES) + (DUMMY_ROW_LOCAL % AGB)

f32 = mybir.dt.float32
f32r = mybir.dt.float32r
f16 = mybir.dt.float16
i16 = mybir.dt.int16
AF = mybir.ActivationFunctionType
ALU = mybir.AluOpType
AX = mybir.AxisListType

TRACE = False          # set by test.py for NTFF profiling
TRACE_KW = {}
LAST_RESULT = {}       # exec_time_ns etc. stashed here when TRACE

_CACHE = {}


def _np_softplus(v):
    v = np.asarray(v, np.float64)
    return np.log1p(np.exp(-np.abs(v))) + np.maximum(v, 0)


def _trow(n):
    """global node id -> table row index (batched-AllGather layout)."""
    c, r = n // NPC, n % NPC
    return (r // AGB) * (AGB * NCORES) + c * AGB + (r % AGB)


def _wrap16(idx, reps=8):
    """idx [n] int -> [16, n/16] wrapped, replicated to [16*reps, n/16]."""
    n = idx.shape[0]
    assert n % 16 == 0
    w = np.ascontiguousarray(idx.reshape(n // 16, 16).T).astype(np.int16)
    return np.tile(w, (reps, 1))


def _preprocess(edge_index, edge_len):
    sender = np.asarray(edge_index[0])
    receiver = np.asarray(edge_index[1])
    el = np.asarray(edge_len, np.float32)
    deg = np.bincount(receiver, minlength=N_NODES)
    assert deg.max() <= 120, "row-major packing assumes deg <= 120"

    order = np.argsort(-deg, kind="stable")
    pos = order[deg[order] > 0]                    # positive-degree nodes
    ntg = (len(pos) + 127) // 128                  # global tiles of 128 rows
    NT = (ntg + NCORES - 1) // NCORES              # slots per core

    # edges grouped by receiver
    eorder = np.argsort(receiver, kind="stable")
    starts = np.searchsorted(receiver[eorder], np.arange(N_NODES))
    ends = np.searchsorted(receiver[eorder], np.arange(N_NODES) + 1)

    # slot depth: max degree within the slot's 8 tiles (sorted -> first tile)
    Ds = []
    for j in range(NT):
        t0 = j * NCORES
        dmax = int(deg[pos[t0 * 128]]) if t0 * 128 < len(pos) else 1
        Ds.append(max(2, dmax + (dmax & 1)))       # even, >= 2

    trow_s = np.array([_trow(int(s)) for s in sender], np.int64)

    cores = []
    for c in range(NCORES):
        idx_parts, len_parts = [], []
        node_of = np.full(NT * 128, -1, np.int64)
        for j in range(NT):
            D = Ds[j]
            g = (j * NCORES + c) * 128             # global rank offset
            rows = pos[g:g + 128]
            idx = np.full((D, 128), DUMMY_TROW, np.int64)
            lem = np.zeros((128, D), np.float32)
            for r, n in enumerate(rows):
                node_of[j * 128 + r] = n
                eids = eorder[starts[n]:ends[n]]
                d = len(eids)
                idx[:d, r] = trow_s[eids]
                lem[r, :d] = el[eids]
            idx_parts.append(idx.reshape(-1))
            len_parts.append(lem)
        flat = np.concatenate(idx_parts)
        cores.append(dict(
            g1i=_wrap16(flat),
            len_pl=np.concatenate(len_parts, axis=1),
            node_of=node_of,
        ))
    return NT, Ds, cores


def _build_program(NT, Ds, w):
    SD = int(np.sum(Ds))
    DMAX = int(max(Ds))
    ms, mb = w["mix_scale"], w["mix_bias"]
    nc = bacc.Bacc("TRN2", target_bir_lowering=False, debug=False,
                   num_devices=NCORES)

    # ---- I/O ----
    xT_d = nc.dram_tensor("xT_in", [F, NPC_PAD], f32r, kind="ExternalInput")
    xpT_d = nc.dram_tensor("xpT", [F, NT * 128], f32r, kind="ExternalInput")
    xperm_d = nc.dram_tensor("x_perm", [NT * 128, F], f32, kind="ExternalInput")
    g1i_d = nc.dram_tensor("g1i", [128, SD * 8], i16, kind="ExternalInput")
    len_d = nc.dram_tensor("len_pl", [128, SD], f32, kind="ExternalInput")
    wrt_d = nc.dram_tensor("wrt", [F, 512], f32r, kind="ExternalInput")
    wuv_d = nc.dram_tensor("wuv2", [F, 8], f32r, kind="ExternalInput")
    dw1_d = nc.dram_tensor("dw1t", [F, H * M], f32r, kind="ExternalInput")
    tw1_d = nc.dram_tensor("tw1t", [F, H * M], f32r, kind="ExternalInput")
    db1_d = nc.dram_tensor("db1c", [H * M, 1], f32, kind="ExternalInput")
    tb1_d = nc.dram_tensor("tb1c", [H * M, 1], f32, kind="ExternalInput")
    dw2_d = nc.dram_tensor("dw2bd", [H * M, H], f32, kind="ExternalInput")
    tw2_d = nc.dram_tensor("tw2bd", [H * M, H], f32, kind="ExternalInput")
    kd_d = nc.dram_tensor("kdc", [128, H], f32, kind="ExternalInput")
    kt_d = nc.dram_tensor("ktc", [128, H], f32, kind="ExternalInput")
    rtw_d = nc.dram_tensor("rtwc", [128, H], f32, kind="ExternalInput")
    wout_d = nc.dram_tensor("wout", [F, F], f32, kind="ExternalInput")
    ident_d = nc.dram_tensor("ident", [128, 128], f32, kind="ExternalInput")
    dummy_d = nc.dram_tensor("dummy_row", [1, ROWW], f16, kind="ExternalInput")
    out_d = nc.dram_tensor("out_perm", [NT * 128, F], f32, kind="ExternalOutput")

    table_shs = [nc.dram_tensor(f"table_sh{b}", [AGB, ROWW], f16)
                 for b in range(NPC_PAD // AGB)]
    table = nc.dram_tensor("table", [TROW_TOT, ROWW], f16, addr_space="Shared")

    with tile.TileContext(nc) as tc:
        # ================= Phase A: table shard =================
        with (
            tc.tile_pool(name="pa_const", bufs=1) as pac,
            tc.tile_pool(name="pa", bufs=3) as pa,
            tc.tile_pool(name="pa_ps", bufs=2, space="PSUM") as pap,
        ):
            wrt = pac.tile([F, 512], f32r)
            nc.sync.dma_start(wrt[:], wrt_d[:])
            wuv = pac.tile([F, 8], f32r)
            nc.sync.dma_start(wuv[:], wuv_d[:])
            xts = pac.tile([F, NPC_PAD], f32r)
            nc.sync.dma_start(xts[:], xT_d[:])
            drow = pac.tile([1, ROWW], f16)
            nc.sync.dma_start(drow[:], dummy_d[:])

            for b in range(NPC_PAD // 128):
                sl = slice(b * 128, (b + 1) * 128)
                ps5 = pap.tile([128, 512], f32, tag="ps5")
                nc.tensor.matmul(ps5[:], xts[:, sl], wrt[:],
                                 start=True, stop=True)
                ps8 = pap.tile([128, 8], f32, tag="ps8")
                nc.tensor.matmul(ps8[:], xts[:, sl].bitcast(f32),
                                 wuv[:].bitcast(f32), start=True, stop=True)
                rt = pa.tile([128, ROWW], f16, tag="rt")
                nc.scalar.copy(rt[:, 0:512], ps5[:])
                # u,v stored as raw f32 bitcast into fp16 slots 512:528
                nc.scalar.copy(rt[:, 512:528].bitcast(f32), ps8[:])
                r0 = b * 128
                bat, boff = r0 // AGB, r0 % AGB
                nrow = 128
                if r0 <= DUMMY_ROW_LOCAL < r0 + 128:
                    nrow = DUMMY_ROW_LOCAL - r0   # leave room for dummy row
                nc.sync.dma_start(table_shs[bat][boff:boff + nrow, :],
                                  rt[0:nrow, :])
            nc.sync.dma_start(
                table_shs[DUMMY_ROW_LOCAL // AGB]
                [DUMMY_ROW_LOCAL % AGB:DUMMY_ROW_LOCAL % AGB + 1, :],
                drow[:])

        # ========= AllGather (overlaps Phase A tail + Phase A2) =========
        for b in range(NPC_PAD // AGB):
            nc.gpsimd.collective_compute(
                "AllGather", ALU.bypass,
                ins=[table_shs[b][:]],
                outs=[table[b * AGB * NCORES:(b + 1) * AGB * NCORES, :]],
                replica_groups=[list(range(NCORES))],
            )

        # ====== Phase A2: receiver-side values from x_perm (local) ======
        with tc.tile_pool(name="pc2", bufs=1) as pc2:
            xpts = pc2.tile([F, NT * 128], f32r)
            nc.sync.dma_start(xpts[:], xpT_d[:])
            wrt2 = pc2.tile([F, 512], f32r)
            nc.sync.dma_start(wrt2[:], wrt_d[:])
            wuv2 = pc2.tile([F, 8], f32r)
            nc.sync.dma_start(wuv2[:], wuv_d[:])
            dw1 = pc2.tile([F, H * M], f32r)
            nc.sync.dma_start(dw1[:], dw1_d[:])
            tw1 = pc2.tile([F, H * M], f32r)
            nc.sync.dma_start(tw1[:], tw1_d[:])
            db1 = pc2.tile([H * M, 1], f32)
            nc.sync.dma_start(db1[:], db1_d[:])
            tb1 = pc2.tile([H * M, 1], f32)
            nc.sync.dma_start(tb1[:], tb1_d[:])
            dw2 = pc2.tile([H * M, H], f32)
            nc.sync.dma_start(dw2[:], dw2_d[:])
            tw2 = pc2.tile([H * M, H], f32)
            nc.sync.dma_start(tw2[:], tw2_d[:])
            kdc = pc2.tile([128, H], f32)
            nc.sync.dma_start(kdc[:], kd_d[:])
            ktc = pc2.tile([128, H], f32)
            nc.sync.dma_start(ktc[:], kt_d[:])
            # persistent per-slot products
            rpo = [pc2.tile([128, 512], f16, tag=f"rpo{s}", name=f"rpo{s}")
                   for s in range(NT)]
            scal = [pc2.tile([128, 12], f32, tag=f"scal{s}", name=f"scal{s}")
                    for s in range(NT)]

            ngrp = (NT * 128 + 511) // 512
            with (
                tc.tile_pool(name="pa2", bufs=2) as pa2,
                tc.tile_pool(name="pa2_ps", bufs=2, space="PSUM") as p2p,
            ):
              for g in range(ngrp):
                gsl = slice(g * 512, min((g + 1) * 512, NT * 128))
                gw = gsl.stop - gsl.start
                # silu via exp + recip (keeps the whole kernel on the
                # natural_log_exp activation table -- no table swaps)
                psh1 = p2p.tile([128, 512], f32, tag="psh1")
                nc.tensor.matmul(psh1[:, 0:gw], dw1[:], xpts[:, gsl],
                                 start=True, stop=True)
                zb1 = pa2.tile([128, 512], f32, tag="zb1")
                nc.vector.tensor_tensor(
                    zb1[:, 0:gw], psh1[:, 0:gw],
                    db1[:].broadcast_to([128, gw]), op=ALU.add)
                h1 = pa2.tile([128, 512], f32, tag="h1")
                nc.scalar.activation(h1[:, 0:gw], zb1[:, 0:gw], AF.Exp,
                                     scale=-1.0)
                nc.scalar.add(h1[:, 0:gw], h1[:, 0:gw], 1.0)
                nc.vector.reciprocal(h1[:, 0:gw], h1[:, 0:gw])
                nc.vector.tensor_tensor(h1[:, 0:gw], h1[:, 0:gw],
                                        zb1[:, 0:gw], op=ALU.mult)
                psh2 = p2p.tile([128, 512], f32, tag="psh1")
                nc.tensor.matmul(psh2[:, 0:gw], tw1[:], xpts[:, gsl],
                                 start=True, stop=True)
                zb2 = pa2.tile([128, 512], f32, tag="zb2")
                nc.vector.tensor_tensor(
                    zb2[:, 0:gw], psh2[:, 0:gw],
                    tb1[:].broadcast_to([128, gw]), op=ALU.add)
                h2 = pa2.tile([128, 512], f32, tag="h2")
                nc.scalar.activation(h2[:, 0:gw], zb2[:, 0:gw], AF.Exp,
                                     scale=-1.0)
                nc.scalar.add(h2[:, 0:gw], h2[:, 0:gw], 1.0)
                nc.vector.reciprocal(h2[:, 0:gw], h2[:, 0:gw])
                nc.vector.tensor_tensor(h2[:, 0:gw], h2[:, 0:gw],
                                        zb2[:, 0:gw], op=ALU.mult)
                for k in range(gw // 128):
                    s = g * 4 + k
                    sl = slice(s * 128, (s + 1) * 128)
                    ks = slice(k * 128, (k + 1) * 128)
                    ps5b = p2p.tile([128, 512], f32, tag="ps5b")
                    nc.tensor.matmul(ps5b[:], xpts[:, sl], wrt2[:],
                                     start=True, stop=True)
                    nc.scalar.copy(rpo[s][:], ps5b[:])
                    psu = p2p.tile([128, 4], f32, tag="psu")
                    nc.tensor.matmul(psu[:], xpts[:, sl].bitcast(f32),
                                     wuv2[:, 0:4].bitcast(f32),
                                     start=True, stop=True)
                    psd = p2p.tile([128, 8], f32, tag="psd")
                    nc.tensor.matmul(psd[:, 0:4], h1[:, ks], dw2[:],
                                     start=True, stop=True)
                    nc.tensor.matmul(psd[:, 4:8], h2[:, ks], tw2[:],
                                     start=True, stop=True)
                    nc.vector.tensor_copy(scal[s][:, 0:4], psu[:])
                    nc.vector.tensor_tensor(scal[s][:, 4:8], psd[:, 0:4],
                                            kdc[:], op=ALU.add)
                    nc.vector.tensor_tensor(scal[s][:, 8:12], psd[:, 4:8],
                                            ktc[:], op=ALU.add)

            # ---- len-only planes, precomputed during the AllGather ----
            g1i = pc2.tile([128, SD * 8], i16)
            nc.sync.dma_start(g1i[:], g1i_d[:])
            lent = pc2.tile([128, SD], f32)
            nc.sync.dma_start(lent[:], len_d[:])
            rtwc = pc2.tile([128, H], f32)
            nc.sync.dma_start(rtwc[:], rtw_d[:])
            GGall = pc2.tile([128, H, SD], f32)
            TMall = pc2.tile([128, H, SD], f32)
            LBall = pc2.tile([128, H, SD], f32)
            for h in range(H):
                nc.scalar.activation(GGall[:, h, :], lent[:], AF.Exp,
                                     scale=float(-ms[h]), bias=float(-mb[h]))
            nc.vector.tensor_scalar_add(GGall[:], GGall[:], 1.0)
            nc.vector.reciprocal(GGall[:], GGall[:])
            lenb_all = lent[:].unsqueeze(1).broadcast_to([128, H, SD])
            nc.vector.tensor_tensor(
                TMall[:], lenb_all,
                rtwc[:].unsqueeze(2).broadcast_to([128, H, SD]), op=ALU.mult)
            off = 0
            for j in range(NT):
                D = Ds[j]
                lsl = slice(off, off + D)
                nc.vector.tensor_tensor(
                    TMall[:, :, lsl], TMall[:, :, lsl],
                    scal[j][:, 8:12].unsqueeze(2).broadcast_to([128, H, D]),
                    op=ALU.add)
                # LB = u_r + (rdscale+doff)*len  (subtracted from u_s later)
                nc.vector.tensor_tensor(
                    LBall[:, :, lsl],
                    lent[:, lsl].unsqueeze(1).broadcast_to([128, H, D]),
                    scal[j][:, 4:8].unsqueeze(2).broadcast_to([128, H, D]),
                    op=ALU.mult)
                nc.vector.tensor_tensor(
                    LBall[:, :, lsl], LBall[:, :, lsl],
                    scal[j][:, 0:4].unsqueeze(2).broadcast_to([128, H, D]),
                    op=ALU.add)
                off += D
            nc.scalar.activation(TMall[:], TMall[:], AF.Exp)
            nc.scalar.activation(TMall[:], TMall[:], AF.Ln, bias=1.0)
            nc.vector.tensor_scalar_add(TMall[:], TMall[:], 1e-4)
            nc.vector.reciprocal(TMall[:], TMall[:])

            # ================= Phase B =================
            with (
                tc.tile_pool(name="pbc", bufs=1) as pbc,
                tc.tile_pool(name="pbg", bufs=2) as pbg,
                tc.tile_pool(name="pf", bufs=3) as pf,
                tc.tile_pool(name="pb_ps", bufs=2, space="PSUM") as pbp,
            ):
                wout = pbc.tile([F, F], f32)
                nc.sync.dma_start(wout[:], wout_d[:])
                ident = pbc.tile([128, 128], f32)
                nc.sync.dma_start(ident[:], ident_d[:])

                off = 0
                for j in range(NT):
                    D = Ds[j]
                    lsl = slice(off, off + D)
                    G = pbg.tile([128, DMAX, ROWW], f16, tag="G", bufs=3)
                    nc.gpsimd.dma_gather(
                        G[:, 0:D, :], table[:, 0:ROWW],
                        g1i[:, off * 8:(off + D) * 8],
                        D * 128, D * 128, elem_size=ROWW, elem_step=ROWW,
                        single_packet=False)

                    # ---- per-edge scalars, [128, H, D] f32 planes ----
                    # u,v of senders live as bitcast f32 in row slots 512:528
                    USv = G[:, 0:D, 512:520].bitcast(f32) \
                        .rearrange("p d h -> p h d")
                    VSv = G[:, 0:D, 520:528].bitcast(f32) \
                        .rearrange("p d h -> p h d")
                    # tangential: exp(v_s - rowmax), sum
                    MX = pf.tile([128, H], f32, tag="MX")
                    nc.vector.tensor_reduce(MX[:], VSv, AX.X, op=ALU.max)
                    LT = pf.tile([128, H, DMAX], f32, tag="LT")
                    nc.vector.tensor_tensor(
                        LT[:, :, 0:D], VSv,
                        MX[:].unsqueeze(2).broadcast_to([128, H, D]),
                        op=ALU.subtract)
                    ET = pf.tile([128, H, DMAX], f32, tag="ET")
                    nc.scalar.activation(ET[:, :, 0:D], LT[:, :, 0:D], AF.Exp)
                    DRT = pf.tile([128, 2 * H], f32, tag="DRT")
                    nc.vector.tensor_reduce(DRT[:, 4:8], ET[:, :, 0:D], AX.X,
                                            op=ALU.add)
                    # radial logits: (u_s - LB) / temp
                    LR = pf.tile([128, H, DMAX], f32, tag="LR")
                    nc.vector.tensor_tensor(LR[:, :, 0:D], USv,
                                            LBall[:, :, lsl], op=ALU.subtract)
                    nc.vector.tensor_tensor(LR[:, :, 0:D], LR[:, :, 0:D],
                                            TMall[:, :, lsl], op=ALU.mult)
                    nc.vector.tensor_reduce(MX[:], LR[:, :, 0:D], AX.X,
                                            op=ALU.max)
                    nc.vector.tensor_tensor(
                        LR[:, :, 0:D], LR[:, :, 0:D],
                        MX[:].unsqueeze(2).broadcast_to([128, H, D]),
                        op=ALU.subtract)
                    ER = pf.tile([128, H, DMAX], f32, tag="ER")
                    nc.scalar.activation(ER[:, :, 0:D], LR[:, :, 0:D], AF.Exp)
                    nc.vector.tensor_reduce(DRT[:, 0:4], ER[:, :, 0:D], AX.X,
                                            op=ALU.add)
                    nc.vector.reciprocal(DRT[:], DRT[:])
                    # alphas and blend weights
                    nc.vector.tensor_tensor(
                        ER[:, :, 0:D], ER[:, :, 0:D],
                        DRT[:, 0:4].unsqueeze(2).broadcast_to([128, H, D]),
                        op=ALU.mult)
                    nc.vector.tensor_tensor(
                        ET[:, :, 0:D], ET[:, :, 0:D],
                        DRT[:, 4:8].unsqueeze(2).broadcast_to([128, H, D]),
                        op=ALU.mult)
                    # BA = g*ra + (1-g)*ta = ta + g*(ra - ta)
                    GGs = GGall[:, :, lsl]
                    W1 = pf.tile([128, H, DMAX], f32, tag="W1")
                    nc.vector.tensor_tensor(W1[:, :, 0:D], ER[:, :, 0:D],
                                            ET[:, :, 0:D], op=ALU.subtract)
                    nc.vector.tensor_tensor(W1[:, :, 0:D], W1[:, :, 0:D],
                                            GGs, op=ALU.mult)
                    nc.vector.tensor_tensor(W1[:, :, 0:D], W1[:, :, 0:D],
                                            ET[:, :, 0:D], op=ALU.add)
                    # W1 <- BA*g ; W2 <- BA - BA*g
                    W2 = pf.tile([128, H, DMAX], f32, tag="W2")
                    nc.vector.tensor_tensor(W2[:, :, 0:D], W1[:, :, 0:D],
                                            GGs, op=ALU.mult)
                    nc.vector.tensor_tensor(W1[:, :, 0:D], W1[:, :, 0:D],
                                            W2[:, :, 0:D], op=ALU.subtract)
                    # now W2 holds BA*g (radial ch), W1 holds BA*(1-g) (tang)
                    C12 = pf.tile([128, 2 * H], f32, tag="C12")
                    nc.vector.tensor_reduce(C12[:, 0:4], W2[:, :, 0:D], AX.X,
                                            op=ALU.add)
                    nc.vector.tensor_reduce(C12[:, 4:8], W1[:, :, 0:D], AX.X,
                                            op=ALU.add)
                    # pack W into fp16 pair-duplicated layout [128, D, 8, 2]
                    WP = pf.tile([128, DMAX, 16], f16, tag="WP")
                    nc.vector.tensor_copy(
                        WP[:, 0:D, 0:8].rearrange("p d (c two) -> p c two d",
                                                  two=2),
                        W2[:, :, 0:D].unsqueeze(2)
                        .broadcast_to([128, H, 2, D]))
                    nc.vector.tensor_copy(
                        WP[:, 0:D, 8:16].rearrange("p d (c two) -> p c two d",
                                                   two=2),
                        W1[:, :, 0:D].unsqueeze(2)
                        .broadcast_to([128, H, 2, D]))
                    C12f = pf.tile([128, 2 * H], f16, tag="C12f")
                    nc.scalar.copy(C12f[:], C12[:])

                    # ---- gating multiply into compact V + flat tree sum ----
                    V = pbg.tile([128, DMAX * 512], f16, tag="V", bufs=1)
                    nc.vector.tensor_tensor(
                        V[:, 0:D * 512].rearrange(
                            "p (d c g2 two) -> p d c g2 two", d=D, c=8, two=2),
                        G[:, 0:D, 0:512].rearrange(
                            "p d (c g2 two) -> p d c g2 two", c=8, two=2),
                        WP[:, 0:D, :].rearrange("p d (c two) -> p d c two",
                                                two=2)
                        .unsqueeze(3).broadcast_to([128, D, 8, 32, 2]),
                        op=ALU.mult)
                    cur = D
                    while cur > 1:
                        nxt = (cur + 1) // 2
                        cnt = cur - nxt
                        nc.vector.tensor_tensor(
                            V[:, 0:cnt * 512], V[:, 0:cnt * 512],
                            V[:, nxt * 512:cur * 512], op=ALU.add)
                        cur = nxt
                    # DIF = Vsum - [C1*rp_r | C2*tp_r]
                    DIF = pf.tile([128, 512], f16, tag="DIF")
                    nc.vector.tensor_tensor(
                        DIF[:].rearrange("p (c f) -> p c f", c=8),
                        rpo[j][:].rearrange("p (c f) -> p c f", c=8),
                        C12f[:].unsqueeze(2).broadcast_to([128, 8, F]),
                        op=ALU.mult)
                    nc.vector.tensor_tensor(DIF[:], V[:, 0:512], DIF[:],
                                            op=ALU.subtract)
                    MMt = pf.tile([128, F], f32, tag="MMt")
                    nc.vector.tensor_reduce(
                        MMt[:], DIF[:].rearrange("p (c f) -> p f c", c=8),
                        AX.X, op=ALU.add)
                    # out = x_perm + (0.25*MM) @ Wout   (0.25 folded in wout)
                    ps_tr = pbp.tile([F, 128], f32, tag="ps_tr")
                    nc.tensor.transpose(ps_tr[:], MMt[:], ident[:])
                    mT = pf.tile([F, 128], f32, tag="mT")
                    nc.scalar.copy(mT[:], ps_tr[:])
                    ps_o = pbp.tile([128, F], f32, tag="ps_o")
                    nc.tensor.matmul(ps_o[:], mT[:], wout[:],
                                     start=True, stop=True)
                    xp = pbg.tile([128, F], f32, tag="xp")
                    nc.sync.dma_start(xp[:], xperm_d[j * 128:(j + 1) * 128, :])
                    ob = pf.tile([128, F], f32, tag="ob")
                    nc.vector.tensor_tensor(ob[:], ps_o[:], xp[:], op=ALU.add)
                    nc.sync.dma_start(out_d[j * 128:(j + 1) * 128, :], ob[:])
                    off += D

    nc.compile()
    return nc


def kernel(**inputs):
    x = np.asarray(inputs["x"], np.float32)
    edge_index = np.asarray(inputs["edge_index"])
    edge_len = np.asarray(inputs["edge_len"], np.float32)

    NT, Ds, cores = _preprocess(edge_index, edge_len)

    ms = np.asarray(inputs["mix_scale"], np.float32)
    mb = np.asarray(inputs["mix_bias"], np.float32)
    w = dict(mix_scale=ms, mix_bias=mb)
    key = (NT, tuple(Ds), ms.tobytes(), mb.tobytes())
    if key not in _CACHE:
        _CACHE[key] = _build_program(NT, Ds, w)
    nc = _CACHE[key]

    Wp = np.asarray(inputs["Wp"], np.float32)
    Wr = np.asarray(inputs["Wr"], np.float32)
    Wt = np.asarray(inputs["Wt"], np.float32)
    rs = np.asarray(inputs["radial_score"], np.float32)
    ts_ = np.asarray(inputs["tangential_score"], np.float32)
    wu = np.einsum("hfg,hg->fh", Wp, rs)                  # [F, H]
    wv = np.einsum("hfg,hg->fh", Wp, ts_)
    wd1f = np.einsum("hfg,hgm->fhm", Wp,
                     np.asarray(inputs["decay_W1"], np.float32))
    wt1f = np.einsum("hfg,hgm->fhm", Wp,
                     np.asarray(inputs["temp_W1"], np.float32))
    dw2bd = np.zeros((H * M, H), np.float32)
    tw2bd = np.zeros((H * M, H), np.float32)
    dw2 = np.asarray(inputs["decay_w2"], np.float32)
    tw2 = np.asarray(inputs["temp_w2"], np.float32)
    for h in range(H):
        dw2bd[h * M:(h + 1) * M, h] = dw2[h]
        tw2bd[h * M:(h + 1) * M, h] = tw2[h]
    kd = (np.asarray(inputs["decay_b2"], np.float64)
          + _np_softplus(inputs["rdls"])).astype(np.float32)
    kt = (np.asarray(inputs["temp_b2"], np.float64)
          + np.asarray(inputs["rtb"], np.float64)).astype(np.float32)
    dummy = np.zeros(ROWW, np.float16)
    # u,v are stored as raw f32 bitcast into fp16 slots 512:528
    dummy[512:528].view(np.float32)[:] = -60000.0
    shared = {
        "wrt": np.ascontiguousarray(np.concatenate(
            [Wr.transpose(1, 0, 2).reshape(F, H * F),
             Wt.transpose(1, 0, 2).reshape(F, H * F)], axis=1)),
        "wuv2": np.ascontiguousarray(np.concatenate([wu, wv], axis=1)),
        "dw1t": np.ascontiguousarray(wd1f.reshape(F, H * M)),
        "tw1t": np.ascontiguousarray(wt1f.reshape(F, H * M)),
        "db1c": np.ascontiguousarray(
            np.asarray(inputs["decay_b1"], np.float32).reshape(H * M, 1)),
        "tb1c": np.ascontiguousarray(
            np.asarray(inputs["temp_b1"], np.float32).reshape(H * M, 1)),
        "dw2bd": dw2bd,
        "tw2bd": tw2bd,
        "kdc": np.tile(kd[None, :], (128, 1)),
        "ktc": np.tile(kt[None, :], (128, 1)),
        "rtwc": np.tile(np.asarray(inputs["rtw"], np.float32)[None, :],
                        (128, 1)),
        "wout": np.ascontiguousarray(
            0.25 * np.asarray(inputs["Wout"], np.float32)),
        "ident": np.eye(128, dtype=np.float32),
        "dummy_row": dummy[None, :],
    }

    in_maps = []
    for c in range(NCORES):
        cc = cores[c]
        xr = np.zeros((F, NPC_PAD), np.float32)
        xr[:, :NPC] = x[c * NPC:(c + 1) * NPC].T
        xp = np.zeros((NT * 128, F), np.float32)
        valid = cc["node_of"] >= 0
        xp[valid] = x[cc["node_of"][valid]]
        in_maps.append(dict(shared, xT_in=xr, x_perm=xp,
                            xpT=np.ascontiguousarray(xp.T),
                            g1i=cc["g1i"], len_pl=cc["len_pl"]))

    r = run_bass_kernel_spmd(nc, in_maps, list(range(NCORES)),
                             trace=TRACE, **TRACE_KW)
    if TRACE:
        LAST_RESULT["exec_time_ns"] = r.exec_time_ns
        LAST_RESULT["mean_exec_time_ns"] = r.mean_exec_time_ns
        LAST_RESULT["raw"] = r

    out = np.array(x, np.float32, copy=True)  # zero-degree nodes: out = x
    for c in range(NCORES):
        cc = cores[c]
        rows = r.results[c]["out_perm"]
        valid = cc["node_of"] >= 0
        out[cc["node_of"][valid]] = rows[valid]
    return out


# revision 25
# speedup vs baseline: 1.3858x; 1.1037x over previous
"""Trainium2 Bass kernel for nn_DenseFlashAttention (GNN message passing).

Design: receivers are degree-sorted and packed 128 per tile-slot with their
edges along the FREE dim (slot depth D = max degree in the slot, ~1% pad).
Segment softmax/sums become native free-dim DVE reduces -- no 0/1 S-matrix
matmuls, no receiver->edge expansion matmuls.  A per-node fp16 table
[rp|tp|u_hi,v_hi|u_lo,v_lo] is built node-range-sharded in Phase A and
AllGathered; Phase B gathers one 1152B row per edge.  Receiver-side values
(u_r, decay/temp offsets, rp_r|tp_r) are computed locally from x_perm in
Phase A2 (overlaps the collective).  The weighted aggregation is an fp16
gating multiply (pair-duplicated W APs keep DVE 2x packing) + in-place
pairwise tree reduction over D.
"""
import numpy as np

import concourse.bacc as bacc
import concourse.mybir as mybir
from concourse import tile
from concourse.bass_utils import run_bass_kernel_spmd

N_NODES = 20000
N_EDGES = 200000
F = 64
H = 4
M = 32
NCORES = 8
NPC = N_NODES // NCORES           # 2500 nodes per core (range shard)
NPC_PAD = 2560                    # padded to 20 x 128
ROWW = 640                        # fp16 elems per table row (1280B, %256==0)
TROW_TOT = NPC_PAD * NCORES       # 20480
BATS = [512, 2048]                # collective batch row counts (batch 0 small
BSTARTS = [0, 512]                # so the AllGather starts early)
DUMMY_ROW_LOCAL = 2548            # shard row (core 0) holding the dummy row
DUMMY_TROW = (BSTARTS[1] * NCORES + 0 * BATS[1]
              + (DUMMY_ROW_LOCAL - BSTARTS[1]))

f32 = mybir.dt.float32
f32r = mybir.dt.float32r
f16 = mybir.dt.float16
i16 = mybir.dt.int16
AF = mybir.ActivationFunctionType
ALU = mybir.AluOpType
AX = mybir.AxisListType

TRACE = False          # set by test.py for NTFF profiling
TRACE_KW = {}
LAST_RESULT = {}       # exec_time_ns etc. stashed here when TRACE

_CACHE = {}


def _np_softplus(v):
    v = np.asarray(v, np.float64)
    return np.log1p(np.exp(-np.abs(v))) + np.maximum(v, 0)


def _trow(n):
    """global node id -> table row index (batched-AllGather layout)."""
    c, r = n // NPC, n % NPC
    k = 0 if r < BSTARTS[1] else 1
    return BSTARTS[k] * NCORES + c * BATS[k] + (r - BSTARTS[k])


def _wrap16(idx, reps=8):
    """idx [n] int -> [16, n/16] wrapped, replicated to [16*reps, n/16]."""
    n = idx.shape[0]
    assert n % 16 == 0
    w = np.ascontiguousarray(idx.reshape(n // 16, 16).T).astype(np.int16)
    return np.tile(w, (reps, 1))


def _preprocess(edge_index, edge_len):
    sender = np.asarray(edge_index[0])
    receiver = np.asarray(edge_index[1])
    el = np.asarray(edge_len, np.float32)
    deg = np.bincount(receiver, minlength=N_NODES)
    assert deg.max() <= 120, "row-major packing assumes deg <= 120"

    order = np.argsort(-deg, kind="stable")
    pos = order[deg[order] > 0]                    # positive-degree nodes
    ntg = (len(pos) + 127) // 128                  # global tiles of 128 rows
    NT = (ntg + NCORES - 1) // NCORES              # slots per core

    # edges grouped by receiver
    eorder = np.argsort(receiver, kind="stable")
    starts = np.searchsorted(receiver[eorder], np.arange(N_NODES))
    ends = np.searchsorted(receiver[eorder], np.arange(N_NODES) + 1)

    # slot depth: max degree within the slot's 8 tiles (sorted -> first tile)
    Ds = []
    for j in range(NT):
        t0 = j * NCORES
        dmax = int(deg[pos[t0 * 128]]) if t0 * 128 < len(pos) else 1
        Ds.append(max(2, dmax + (dmax & 1)))       # even, >= 2

    trow_s = np.array([_trow(int(s)) for s in sender], np.int64)

    cores = []
    for c in range(NCORES):
        idx_parts, len_parts = [], []
        node_of = np.full(NT * 128, -1, np.int64)
        for j in range(NT):
            D = Ds[j]
            g = (j * NCORES + c) * 128             # global rank offset
            rows = pos[g:g + 128]
            idx = np.full((D, 128), DUMMY_TROW, np.int64)
            lem = np.zeros((128, D), np.float32)
            for r, n in enumerate(rows):
                node_of[j * 128 + r] = n
                eids = eorder[starts[n]:ends[n]]
                d = len(eids)
                idx[:d, r] = trow_s[eids]
                lem[r, :d] = el[eids]
            idx_parts.append(idx.reshape(-1))
            len_parts.append(lem)
        flat = np.concatenate(idx_parts)
        cores.append(dict(
            g1i=_wrap16(flat),
            len_pl=np.concatenate(len_parts, axis=1),
            node_of=node_of,
        ))
    return NT, Ds, cores


def _build_program(NT, Ds, w):
    SD = int(np.sum(Ds))
    DMAX = int(max(Ds))
    ms, mb = w["mix_scale"], w["mix_bias"]
    nc = bacc.Bacc("TRN2", target_bir_lowering=False, debug=False,
                   num_devices=NCORES)

    # ---- I/O ----
    xT_d = nc.dram_tensor("xT_in", [F, NPC_PAD], f32r, kind="ExternalInput")
    xpT_d = nc.dram_tensor("xpT", [F, NT * 128], f32r, kind="ExternalInput")
    xperm_d = nc.dram_tensor("x_perm", [NT * 128, F], f32, kind="ExternalInput")
    g1i_d = nc.dram_tensor("g1i", [128, SD * 8], i16, kind="ExternalInput")
    len_d = nc.dram_tensor("len_pl", [128, SD], f32, kind="ExternalInput")
    wrt_d = nc.dram_tensor("wrt", [F, 512], f32r, kind="ExternalInput")
    wuv_d = nc.dram_tensor("wuv2", [F, 8], f32r, kind="ExternalInput")
    dw1_d = nc.dram_tensor("dw1t", [F, H * M], f32r, kind="ExternalInput")
    tw1_d = nc.dram_tensor("tw1t", [F, H * M], f32r, kind="ExternalInput")
    db1_d = nc.dram_tensor("db1c", [H * M, 1], f32, kind="ExternalInput")
    tb1_d = nc.dram_tensor("tb1c", [H * M, 1], f32, kind="ExternalInput")
    dw2_d = nc.dram_tensor("dw2bd", [H * M, H], f32, kind="ExternalInput")
    tw2_d = nc.dram_tensor("tw2bd", [H * M, H], f32, kind="ExternalInput")
    kd_d = nc.dram_tensor("kdc", [128, H], f32, kind="ExternalInput")
    kt_d = nc.dram_tensor("ktc", [128, H], f32, kind="ExternalInput")
    rtw_d = nc.dram_tensor("rtwc", [128, H], f32, kind="ExternalInput")
    wout_d = nc.dram_tensor("wout", [F, F], f32, kind="ExternalInput")
    ident_d = nc.dram_tensor("ident", [128, 128], f32, kind="ExternalInput")
    dummy_d = nc.dram_tensor("dummy_row", [1, ROWW], f16, kind="ExternalInput")
    out_d = nc.dram_tensor("out_perm", [NT * 128, F], f32, kind="ExternalOutput")

    table_shs = [nc.dram_tensor(f"table_sh{b}", [BATS[b], ROWW], f16)
                 for b in range(len(BATS))]
    table = nc.dram_tensor("table", [TROW_TOT, ROWW], f16, addr_space="Shared")

    with tile.TileContext(nc) as tc:
        # ================= Phase A: table shard =================
        with (
            tc.tile_pool(name="pa_const", bufs=1) as pac,
            tc.tile_pool(name="pa", bufs=3) as pa,
            tc.tile_pool(name="pa_ps", bufs=2, space="PSUM") as pap,
        ):
            wrt = pac.tile([F, 512], f32r)
            nc.sync.dma_start(wrt[:], wrt_d[:])
            wuv = pac.tile([F, 8], f32r)
            nc.sync.dma_start(wuv[:], wuv_d[:])
            xts = pac.tile([F, NPC_PAD], f32r)
            for q in range(4):
                qs = slice(q * NPC_PAD // 4, (q + 1) * NPC_PAD // 4)
                nc.sync.dma_start(xts[:, qs], xT_d[:, qs])
            drow = pac.tile([1, ROWW], f16)
            nc.sync.dma_start(drow[:], dummy_d[:])

            for b in range(NPC_PAD // 128):
                sl = slice(b * 128, (b + 1) * 128)
                ps5 = pap.tile([128, 512], f32, tag="ps5")
                nc.tensor.matmul(ps5[:], xts[:, sl], wrt[:],
                                 start=True, stop=True)
                ps8 = pap.tile([128, 8], f32, tag="ps8")
                nc.tensor.matmul(ps8[:], xts[:, sl].bitcast(f32),
                                 wuv[:].bitcast(f32), start=True, stop=True)
                rt = pa.tile([128, ROWW], f16, tag="rt")
                nc.scalar.copy(rt[:, 0:512], ps5[:])
                # u,v stored as raw f32 bitcast into fp16 slots 512:528
                nc.scalar.copy(rt[:, 512:528].bitcast(f32), ps8[:])
                r0 = b * 128
                bat = 0 if r0 < BSTARTS[1] else 1
                boff = r0 - BSTARTS[bat]
                nrow = 128
                if r0 <= DUMMY_ROW_LOCAL < r0 + 128:
                    nrow = DUMMY_ROW_LOCAL - r0   # leave room for dummy row
                nc.sync.dma_start(table_shs[bat][boff:boff + nrow, :],
                                  rt[0:nrow, :])
            dumo = DUMMY_ROW_LOCAL - BSTARTS[1]
            nc.sync.dma_start(table_shs[1][dumo:dumo + 1, :], drow[:])

        # ========= AllGather (overlaps Phase A tail + Phase A2) =========
        for b in range(len(BATS)):
            t0r = BSTARTS[b] * NCORES
            nc.gpsimd.collective_compute(
                "AllGather", ALU.bypass,
                ins=[table_shs[b][:]],
                outs=[table[t0r:t0r + BATS[b] * NCORES, :]],
                replica_groups=[list(range(NCORES))],
            )

        # ====== Phase A2: receiver-side values from x_perm (local) ======
        with tc.tile_pool(name="pc2", bufs=1) as pc2:
            xpts = pc2.tile([F, NT * 128], f32r)
            for q in range(4):
                qs = slice(q * NT * 128 // 4, (q + 1) * NT * 128 // 4)
                nc.sync.dma_start(xpts[:, qs], xpT_d[:, qs])
            wrt2 = pc2.tile([F, 512], f32r)
            nc.sync.dma_start(wrt2[:], wrt_d[:])
            wuv2 = pc2.tile([F, 8], f32r)
            nc.sync.dma_start(wuv2[:], wuv_d[:])
            dw1 = pc2.tile([F, H * M], f32r)
            nc.sync.dma_start(dw1[:], dw1_d[:])
            tw1 = pc2.tile([F, H * M], f32r)
            nc.sync.dma_start(tw1[:], tw1_d[:])
            db1 = pc2.tile([H * M, 1], f32)
            nc.sync.dma_start(db1[:], db1_d[:])
            tb1 = pc2.tile([H * M, 1], f32)
            nc.sync.dma_start(tb1[:], tb1_d[:])
            dw2 = pc2.tile([H * M, H], f32)
            nc.sync.dma_start(dw2[:], dw2_d[:])
            tw2 = pc2.tile([H * M, H], f32)
            nc.sync.dma_start(tw2[:], tw2_d[:])
            kdc = pc2.tile([128, H], f32)
            nc.sync.dma_start(kdc[:], kd_d[:])
            ktc = pc2.tile([128, H], f32)
            nc.sync.dma_start(ktc[:], kt_d[:])
            # persistent per-slot products
            rpo = [pc2.tile([128, 512], f16, tag=f"rpo{s}", name=f"rpo{s}")
                   for s in range(NT)]
            scal = [pc2.tile([128, 12], f32, tag=f"scal{s}", name=f"scal{s}")
                    for s in range(NT)]

            ngrp = (NT * 128 + 511) // 512
            with (
                tc.tile_pool(name="pa2", bufs=2) as pa2,
                tc.tile_pool(name="pa2_ps", bufs=2, space="PSUM") as p2p,
            ):
              for g in range(ngrp):
                gsl = slice(g * 512, min((g + 1) * 512, NT * 128))
                gw = gsl.stop - gsl.start
                # silu via exp + recip (keeps the whole kernel on the
                # natural_log_exp activation table -- no table swaps)
                psh1 = p2p.tile([128, 512], f32, tag="psh1")
                nc.tensor.matmul(psh1[:, 0:gw], dw1[:], xpts[:, gsl],
                                 start=True, stop=True)
                zb1 = pa2.tile([128, 512], f32, tag="zb1")
                nc.vector.tensor_tensor(
                    zb1[:, 0:gw], psh1[:, 0:gw],
                    db1[:].broadcast_to([128, gw]), op=ALU.add)
                h1 = pa2.tile([128, 512], f32, tag="h1")
                nc.scalar.activation(h1[:, 0:gw], zb1[:, 0:gw], AF.Exp,
                                     scale=-1.0)
                nc.scalar.add(h1[:, 0:gw], h1[:, 0:gw], 1.0)
                nc.vector.reciprocal(h1[:, 0:gw], h1[:, 0:gw])
                nc.vector.tensor_tensor(h1[:, 0:gw], h1[:, 0:gw],
                                        zb1[:, 0:gw], op=ALU.mult)
                psh2 = p2p.tile([128, 512], f32, tag="psh1")
                nc.tensor.matmul(psh2[:, 0:gw], tw1[:], xpts[:, gsl],
                                 start=True, stop=True)
                zb2 = pa2.tile([128, 512], f32, tag="zb2")
                nc.vector.tensor_tensor(
                    zb2[:, 0:gw], psh2[:, 0:gw],
                    tb1[:].broadcast_to([128, gw]), op=ALU.add)
                h2 = pa2.tile([128, 512], f32, tag="h2")
                nc.scalar.activation(h2[:, 0:gw], zb2[:, 0:gw], AF.Exp,
                                     scale=-1.0)
                nc.scalar.add(h2[:, 0:gw], h2[:, 0:gw], 1.0)
                nc.vector.reciprocal(h2[:, 0:gw], h2[:, 0:gw])
                nc.vector.tensor_tensor(h2[:, 0:gw], h2[:, 0:gw],
                                        zb2[:, 0:gw], op=ALU.mult)
                for k in range(gw // 128):
                    s = g * 4 + k
                    sl = slice(s * 128, (s + 1) * 128)
                    ks = slice(k * 128, (k + 1) * 128)
                    ps5b = p2p.tile([128, 512], f32, tag="ps5b")
                    nc.tensor.matmul(ps5b[:], xpts[:, sl], wrt2[:],
                                     start=True, stop=True)
                    nc.scalar.copy(rpo[s][:], ps5b[:])
                    psu = p2p.tile([128, 4], f32, tag="psu")
                    nc.tensor.matmul(psu[:], xpts[:, sl].bitcast(f32),
                                     wuv2[:, 0:4].bitcast(f32),
                                     start=True, stop=True)
                    psd = p2p.tile([128, 8], f32, tag="psd")
                    nc.tensor.matmul(psd[:, 0:4], h1[:, ks], dw2[:],
                                     start=True, stop=True)
                    nc.tensor.matmul(psd[:, 4:8], h2[:, ks], tw2[:],
                                     start=True, stop=True)
                    nc.vector.tensor_copy(scal[s][:, 0:4], psu[:])
                    nc.vector.tensor_tensor(scal[s][:, 4:8], psd[:, 0:4],
                                            kdc[:], op=ALU.add)
                    nc.vector.tensor_tensor(scal[s][:, 8:12], psd[:, 4:8],
                                            ktc[:], op=ALU.add)

            # ---- len-only planes, precomputed during the AllGather ----
            g1i = pc2.tile([128, SD * 8], i16)
            nc.sync.dma_start(g1i[:], g1i_d[:])
            lent = pc2.tile([128, SD], f32)
            nc.sync.dma_start(lent[:], len_d[:])
            rtwc = pc2.tile([128, H], f32)
            nc.sync.dma_start(rtwc[:], rtw_d[:])
            GGall = pc2.tile([128, H, SD], f32)
            TMall = pc2.tile([128, H, SD], f32)
            LBall = pc2.tile([128, H, SD], f32)
            for h in range(H):
                nc.scalar.activation(GGall[:, h, :], lent[:], AF.Exp,
                                     scale=float(-ms[h]), bias=float(-mb[h]))
            nc.vector.tensor_scalar_add(GGall[:], GGall[:], 1.0)
            nc.vector.reciprocal(GGall[:], GGall[:])
            lenb_all = lent[:].unsqueeze(1).broadcast_to([128, H, SD])
            nc.vector.tensor_tensor(
                TMall[:], lenb_all,
                rtwc[:].unsqueeze(2).broadcast_to([128, H, SD]), op=ALU.mult)
            off = 0
            for j in range(NT):
                D = Ds[j]
                lsl = slice(off, off + D)
                nc.vector.tensor_tensor(
                    TMall[:, :, lsl], TMall[:, :, lsl],
                    scal[j][:, 8:12].unsqueeze(2).broadcast_to([128, H, D]),
                    op=ALU.add)
                # LB = u_r + (rdscale+doff)*len  (subtracted from u_s later)
                nc.vector.tensor_tensor(
                    LBall[:, :, lsl],
                    lent[:, lsl].unsqueeze(1).broadcast_to([128, H, D]),
                    scal[j][:, 4:8].unsqueeze(2).broadcast_to([128, H, D]),
                    op=ALU.mult)
                nc.vector.tensor_tensor(
                    LBall[:, :, lsl], LBall[:, :, lsl],
                    scal[j][:, 0:4].unsqueeze(2).broadcast_to([128, H, D]),
                    op=ALU.add)
                off += D
            nc.scalar.activation(TMall[:], TMall[:], AF.Exp)
            nc.scalar.activation(TMall[:], TMall[:], AF.Ln, bias=1.0)
            nc.vector.tensor_scalar_add(TMall[:], TMall[:], 1e-4)
            nc.vector.reciprocal(TMall[:], TMall[:])

            # ================= Phase B =================
            with (
                tc.tile_pool(name="pbc", bufs=1) as pbc,
                tc.tile_pool(name="pbg", bufs=2) as pbg,
                tc.tile_pool(name="pf", bufs=3) as pf,
                tc.tile_pool(name="pb_ps", bufs=2, space="PSUM") as pbp,
            ):
                wout = pbc.tile([F, F], f32)
                nc.sync.dma_start(wout[:], wout_d[:])
                ident = pbc.tile([128, 128], f32)
                nc.sync.dma_start(ident[:], ident_d[:])

                off = 0
                for j in range(NT):
                    D = Ds[j]
                    lsl = slice(off, off + D)
                    G = pbg.tile([128, DMAX, ROWW], f16, tag="G", bufs=3)
                    nc.gpsimd.dma_gather(
                        G[:, 0:D, :], table[:, 0:ROWW],
                        g1i[:, off * 8:(off + D) * 8],
                        D * 128, D * 128, elem_size=ROWW, elem_step=ROWW,
                        single_packet=False)

                    # ---- per-edge scalars, [128, H, D] f32 planes ----
                    # u,v of senders live as bitcast f32 in row slots 512:528
                    USv = G[:, 0:D, 512:520].bitcast(f32) \
                        .rearrange("p d h -> p h d")
                    VSv = G[:, 0:D, 520:528].bitcast(f32) \
                        .rearrange("p d h -> p h d")
                    # tangential: exp(v_s - rowmax), sum
                    MX = pf.tile([128, H], f32, tag="MX")
                    nc.vector.tensor_reduce(MX[:], VSv, AX.X, op=ALU.max)
                    LT = pf.tile([128, H, DMAX], f32, tag="LT")
                    nc.vector.tensor_tensor(
                        LT[:, :, 0:D], VSv,
                        MX[:].unsqueeze(2).broadcast_to([128, H, D]),
                        op=ALU.subtract)
                    ET = pf.tile([128, H, DMAX], f32, tag="ET")
                    nc.scalar.activation(ET[:, :, 0:D], LT[:, :, 0:D], AF.Exp)
                    DRT = pf.tile([128, 2 * H], f32, tag="DRT")
                    nc.vector.tensor_reduce(DRT[:, 4:8], ET[:, :, 0:D], AX.X,
                                            op=ALU.add)
                    # radial logits: (u_s - LB) / temp
                    LR = pf.tile([128, H, DMAX], f32, tag="LR")
                    nc.vector.tensor_tensor(LR[:, :, 0:D], USv,
                                            LBall[:, :, lsl], op=ALU.subtract)
                    nc.vector.tensor_tensor(LR[:, :, 0:D], LR[:, :, 0:D],
                                            TMall[:, :, lsl], op=ALU.mult)
                    nc.vector.tensor_reduce(MX[:], LR[:, :, 0:D], AX.X,
                                            op=ALU.max)
                    nc.vector.tensor_tensor(
                        LR[:, :, 0:D], LR[:, :, 0:D],
                        MX[:].unsqueeze(2).broadcast_to([128, H, D]),
                        op=ALU.subtract)
                    ER = pf.tile([128, H, DMAX], f32, tag="ER")
                    nc.scalar.activation(ER[:, :, 0:D], LR[:, :, 0:D], AF.Exp)
                    nc.vector.tensor_reduce(DRT[:, 0:4], ER[:, :, 0:D], AX.X,
                                            op=ALU.add)
                    nc.vector.reciprocal(DRT[:], DRT[:])
                    # alphas and blend weights
                    nc.vector.tensor_tensor(
                        ER[:, :, 0:D], ER[:, :, 0:D],
                        DRT[:, 0:4].unsqueeze(2).broadcast_to([128, H, D]),
                        op=ALU.mult)
                    nc.vector.tensor_tensor(
                        ET[:, :, 0:D], ET[:, :, 0:D],
                        DRT[:, 4:8].unsqueeze(2).broadcast_to([128, H, D]),
                        op=ALU.mult)
                    # BA = g*ra + (1-g)*ta = ta + g*(ra - ta)
                    GGs = GGall[:, :, lsl]
                    W1 = pf.tile([128, H, DMAX], f32, tag="W1")
                    nc.vector.tensor_tensor(W1[:, :, 0:D], ER[:, :, 0:D],
                                            ET[:, :, 0:D], op=ALU.subtract)
                    nc.vector.tensor_tensor(W1[:, :, 0:D], W1[:, :, 0:D],
                                            GGs, op=ALU.mult)
                    nc.vector.tensor_tensor(W1[:, :, 0:D], W1[:, :, 0:D],
                                            ET[:, :, 0:D], op=ALU.add)
                    # W1 <- BA*g ; W2 <- BA - BA*g
                    W2 = pf.tile([128, H, DMAX], f32, tag="W2")
                    nc.vector.tensor_tensor(W2[:, :, 0:D], W1[:, :, 0:D],
                                            GGs, op=ALU.mult)
                    nc.vector.tensor_tensor(W1[:, :, 0:D], W1[:, :, 0:D],
                                            W2[:, :, 0:D], op=ALU.subtract)
                    # now W2 holds BA*g (radial ch), W1 holds BA*(1-g) (tang)
                    C12 = pf.tile([128, 2 * H], f32, tag="C12")
                    nc.vector.tensor_reduce(C12[:, 0:4], W2[:, :, 0:D], AX.X,
                                            op=ALU.add)
                    nc.vector.tensor_reduce(C12[:, 4:8], W1[:, :, 0:D], AX.X,
                                            op=ALU.add)
                    # pack W into fp16 pair-duplicated layout [128, D, 8, 2]
                    WP = pf.tile([128, DMAX, 16], f16, tag="WP")
                    nc.vector.tensor_copy(
                        WP[:, 0:D, 0:8].rearrange("p d (c two) -> p c two d",
                                                  two=2),
                        W2[:, :, 0:D].unsqueeze(2)
                        .broadcast_to([128, H, 2, D]))
                    nc.vector.tensor_copy(
                        WP[:, 0:D, 8:16].rearrange("p d (c two) -> p c two d",
                                                   two=2),
                        W1[:, :, 0:D].unsqueeze(2)
                        .broadcast_to([128, H, 2, D]))
                    C12f = pf.tile([128, 2 * H], f16, tag="C12f")
                    nc.scalar.copy(C12f[:], C12[:])

                    # ---- gating multiply into compact V + flat tree sum ----
                    V = pbg.tile([128, DMAX * 512], f16, tag="V", bufs=1)
                    nc.vector.tensor_tensor(
                        V[:, 0:D * 512].rearrange(
                            "p (d c g2 two) -> p d c g2 two", d=D, c=8, two=2),
                        G[:, 0:D, 0:512].rearrange(
                            "p d (c g2 two) -> p d c g2 two", c=8, two=2),
                        WP[:, 0:D, :].rearrange("p d (c two) -> p d c two",
                                                two=2)
                        .unsqueeze(3).broadcast_to([128, D, 8, 32, 2]),
                        op=ALU.mult)
                    cur = D
                    while cur > 1:
                        nxt = (cur + 1) // 2
                        cnt = cur - nxt
                        nc.vector.tensor_tensor(
                            V[:, 0:cnt * 512], V[:, 0:cnt * 512],
                            V[:, nxt * 512:cur * 512], op=ALU.add)
                        cur = nxt
                    # DIF = Vsum - [C1*rp_r | C2*tp_r]
                    DIF = pf.tile([128, 512], f16, tag="DIF")
                    nc.vector.tensor_tensor(
                        DIF[:].rearrange("p (c f) -> p c f", c=8),
                        rpo[j][:].rearrange("p (c f) -> p c f", c=8),
                        C12f[:].unsqueeze(2).broadcast_to([128, 8, F]),
                        op=ALU.mult)
                    nc.vector.tensor_tensor(DIF[:], V[:, 0:512], DIF[:],
                                            op=ALU.subtract)
                    MMt = pf.tile([128, F], f32, tag="MMt")
                    nc.vector.tensor_reduce(
                        MMt[:], DIF[:].rearrange("p (c f) -> p f c", c=8),
                        AX.X, op=ALU.add)
                    # out = x_perm + (0.25*MM) @ Wout   (0.25 folded in wout)
                    ps_tr = pbp.tile([F, 128], f32, tag="ps_tr")
                    nc.tensor.transpose(ps_tr[:], MMt[:], ident[:])
                    mT = pf.tile([F, 128], f32, tag="mT")
                    nc.scalar.copy(mT[:], ps_tr[:])
                    ps_o = pbp.tile([128, F], f32, tag="ps_o")
                    nc.tensor.matmul(ps_o[:], mT[:], wout[:],
                                     start=True, stop=True)
                    xp = pbg.tile([128, F], f32, tag="xp")
                    nc.sync.dma_start(xp[:], xperm_d[j * 128:(j + 1) * 128, :])
                    ob = pf.tile([128, F], f32, tag="ob")
                    nc.vector.tensor_tensor(ob[:], ps_o[:], xp[:], op=ALU.add)
                    nc.sync.dma_start(out_d[j * 128:(j + 1) * 128, :], ob[:])
                    off += D

    nc.compile()
    return nc


def kernel(**inputs):
    x = np.asarray(inputs["x"], np.float32)
    edge_index = np.asarray(inputs["edge_index"])
    edge_len = np.asarray(inputs["edge_len"], np.float32)

    NT, Ds, cores = _preprocess(edge_index, edge_len)

    ms = np.asarray(inputs["mix_scale"], np.float32)
    mb = np.asarray(inputs["mix_bias"], np.float32)
    w = dict(mix_scale=ms, mix_bias=mb)
    key = (NT, tuple(Ds), ms.tobytes(), mb.tobytes())
    if key not in _CACHE:
        _CACHE[key] = _build_program(NT, Ds, w)
    nc = _CACHE[key]

    Wp = np.asarray(inputs["Wp"], np.float32)
    Wr = np.asarray(inputs["Wr"], np.float32)
    Wt = np.asarray(inputs["Wt"], np.float32)
    rs = np.asarray(inputs["radial_score"], np.float32)
    ts_ = np.asarray(inputs["tangential_score"], np.float32)
    wu = np.einsum("hfg,hg->fh", Wp, rs)                  # [F, H]
    wv = np.einsum("hfg,hg->fh", Wp, ts_)
    wd1f = np.einsum("hfg,hgm->fhm", Wp,
                     np.asarray(inputs["decay_W1"], np.float32))
    wt1f = np.einsum("hfg,hgm->fhm", Wp,
                     np.asarray(inputs["temp_W1"], np.float32))
    dw2bd = np.zeros((H * M, H), np.float32)
    tw2bd = np.zeros((H * M, H), np.float32)
    dw2 = np.asarray(inputs["decay_w2"], np.float32)
    tw2 = np.asarray(inputs["temp_w2"], np.float32)
    for h in range(H):
        dw2bd[h * M:(h + 1) * M, h] = dw2[h]
        tw2bd[h * M:(h + 1) * M, h] = tw2[h]
    kd = (np.asarray(inputs["decay_b2"], np.float64)
          + _np_softplus(inputs["rdls"])).astype(np.float32)
    kt = (np.asarray(inputs["temp_b2"], np.float64)
          + np.asarray(inputs["rtb"], np.float64)).astype(np.float32)
    dummy = np.zeros(ROWW, np.float16)
    # u,v are stored as raw f32 bitcast into fp16 slots 512:528
    dummy[512:528].view(np.float32)[:] = -60000.0
    shared = {
        "wrt": np.ascontiguousarray(np.concatenate(
            [Wr.transpose(1, 0, 2).reshape(F, H * F),
             Wt.transpose(1, 0, 2).reshape(F, H * F)], axis=1)),
        "wuv2": np.ascontiguousarray(np.concatenate([wu, wv], axis=1)),
        "dw1t": np.ascontiguousarray(wd1f.reshape(F, H * M)),
        "tw1t": np.ascontiguousarray(wt1f.reshape(F, H * M)),
        "db1c": np.ascontiguousarray(
            np.asarray(inputs["decay_b1"], np.float32).reshape(H * M, 1)),
        "tb1c": np.ascontiguousarray(
            np.asarray(inputs["temp_b1"], np.float32).reshape(H * M, 1)),
        "dw2bd": dw2bd,
        "tw2bd": tw2bd,
        "kdc": np.tile(kd[None, :], (128, 1)),
        "ktc": np.tile(kt[None, :], (128, 1)),
        "rtwc": np.tile(np.asarray(inputs["rtw"], np.float32)[None, :],
                        (128, 1)),
        "wout": np.ascontiguousarray(
            0.25 * np.asarray(inputs["Wout"], np.float32)),
        "ident": np.eye(128, dtype=np.float32),
        "dummy_row": dummy[None, :],
    }

    in_maps = []
    for c in range(NCORES):
        cc = cores[c]
        xr = np.zeros((F, NPC_PAD), np.float32)
        xr[:, :NPC] = x[c * NPC:(c + 1) * NPC].T
        xp = np.zeros((NT * 128, F), np.float32)
        valid = cc["node_of"] >= 0
        xp[valid] = x[cc["node_of"][valid]]
        in_maps.append(dict(shared, xT_in=xr, x_perm=xp,
                            xpT=np.ascontiguousarray(xp.T),
                            g1i=cc["g1i"], len_pl=cc["len_pl"]))

    r = run_bass_kernel_spmd(nc, in_maps, list(range(NCORES)),
                             trace=TRACE, **TRACE_KW)
    if TRACE:
        LAST_RESULT["exec_time_ns"] = r.exec_time_ns
        LAST_RESULT["mean_exec_time_ns"] = r.mean_exec_time_ns
        LAST_RESULT["raw"] = r

    out = np.array(x, np.float32, copy=True)  # zero-degree nodes: out = x
    for c in range(NCORES):
        cc = cores[c]
        rows = r.results[c]["out_perm"]
        valid = cc["node_of"] >= 0
        out[cc["node_of"][valid]] = rows[valid]
    return out
